# revision 26
# baseline (speedup 1.0000x reference)
"""Causal self-attention (B=2, T=2048, C=1024, H=16) on 8 trn2 NeuronCores.

Sharding: core c = (b, g) with b = c // 4 (batch), g = c % 4 (head-group of 4
heads = 256 dims).  No collectives: each core computes a PARTIAL output
projection over its own 256 head-dims (o_part^T = W_g^T y_g^T, bf16) and the
host sums the 4 partials per batch during unsharding.

v3 schedule: the attention phases are ScalarE(exp)-bound, so QKV / output
projection matmul groups are emitted as "fillers" interleaved INTO the
attention kt-loops -- the in-order PE queue always has independent ready work
while exp lags.  Fillers are paced away from the p-boundaries so the Vector
FIFO drains before each normalize (whose PSUM reads gate the y-bank handoff
to the next p half).  Input DMAs are merged (w_qkv as one [C,768] tensor,
biases packed) to cut the ~0.7us/instruction descriptor-generation cost, and
alternate between the two hardware DGE queues in consumption order.
"""
import math

import numpy as np
import ml_dtypes

B, T, C, H = 2, 2048, 1024, 16
HD = C // H          # 64 head dim
G = 4                # head-groups (cores per batch)
HPG = H // G         # 4 heads per group
DG = HPG * HD        # 256 dims per group
N_CORES = 8
KC = C // 128        # 8 contraction chunks
NTC = T // 512       # 4 t-chunks (and attention q-chunks)
VW = HD + 2          # V1 per-head stride (64 data + 1 ones + 1 pad)

_NC_CACHE = {}


def _build():
    import concourse.bacc as bacc
    import concourse.mybir as mybir
    import concourse.tile as tile

    f32 = mybir.dt.float32
    f32r = mybir.dt.float32r
    bf16 = mybir.dt.bfloat16
    Exp = mybir.ActivationFunctionType.Exp
    Ident = mybir.ActivationFunctionType.Identity
    Copy = mybir.ActivationFunctionType.Copy

    nc = bacc.Bacc("TRN2", num_devices=N_CORES)

    xT_d = nc.dram_tensor("xT", [C, T], bf16, kind="ExternalInput")
    # merged [wq | wk] columns, pre-transposed: [C, 2*DG]; wv separate
    wqk_d = nc.dram_tensor("wqk", [C, 2 * DG], bf16, kind="ExternalInput")
    wv_d = nc.dram_tensor("wv", [C, DG], bf16, kind="ExternalInput")
    # packed biases: cols = [bq jh0, bq jh1, bk jh0, bk jh1]
    bqk_d = nc.dram_tensor("bqk", [128, 4], f32, kind="ExternalInput")
    bv_d = nc.dram_tensor("bv", [1, DG], f32, kind="ExternalInput")
    # w_proj.T rows [lo:lo+DG] -> partial projection stationary [DG, C]
    wp_d = nc.dram_tensor("wpT", [DG, C], bf16, kind="ExternalInput")
    mask_d = nc.dram_tensor("mask", [128, 128], bf16, kind="ExternalInput")
    oP_d = nc.dram_tensor("oP", [C, T], bf16, kind="ExternalOutput")

    with tile.TileContext(nc) as tc:
        with (
            tc.tile_pool(name="persist", bufs=1) as persist,
            tc.tile_pool(name="xp", bufs=1) as xp,
            tc.tile_pool(name="wp_s", bufs=1) as wp_s,
            tc.tile_pool(name="psp", bufs=1, space="PSUM") as psp,
            tc.tile_pool(name="ppool", bufs=1) as ppool,
            tc.tile_pool(name="npool", bufs=1) as npool,
            tc.tile_pool(name="ynp", bufs=1) as ynp,
            tc.tile_pool(name="otp", bufs=1) as otp,
        ):
            # ---- persistent SBUF ----
            QT = [[persist.tile([128, 512], f32r, name=f"qt{t}_{j}")
                   for j in range(2)] for t in range(NTC)]
            KT = [[persist.tile([128, 512], f32r, name=f"kt{t}_{j}")
                   for j in range(2)] for t in range(NTC)]
            V1 = [persist.tile([128, HPG * VW], bf16, name=f"v{m}")
                  for m in range(4 * NTC)]
            # t=0,1: one tile per (t, k); t=2,3 merged: one [128,1024] per k
            xT01 = [[xp.tile([128, 512], bf16, name=f"x{t}_{k}")
                     for k in range(KC)] for t in range(2)]
            xT23 = [xp.tile([128, 1024], bf16, name=f"x23_{k}")
                    for k in range(KC)]

            def x_sl(t, k, lo=0, hi=512):
                if t < 2:
                    return xT01[t][k][:, lo:hi]
                return xT23[k][:, 512 * (t - 2) + lo:512 * (t - 2) + hi]
            wqk_sb = [wp_s.tile([128, 2 * DG], bf16, name=f"w{k}")
                      for k in range(KC)]
            wv_sb = [wp_s.tile([128, DG], bf16, name=f"wv{k}")
                     for k in range(KC)]
            wpT_sb = [persist.tile([128, C], bf16, name=f"wp_{k}")
                      for k in range(2)]
            mask_sb = persist.tile([128, 128], bf16, name="mask_sb")
            bqk_sb = persist.tile([128, 4], f32, name="bqk_sb")
            bv_row = persist.tile([1, DG], f32, name="bv_row")
            bv_bc = persist.tile([128, DG], f32, name="bv_bc")

            # ---- prologue: warmup + DMA streaming ----
            wu_a = wp_s.tile([128, 128], bf16, name="wu_a")
            wu_b = wp_s.tile([128, 512], bf16, name="wu_b")
            nc.vector.memset(wu_a[:], 0.5)
            nc.vector.memset(wu_b[:], 0.5)
            for i in range(40):
                wu_ps = psp.tile([128, 512], f32, tag="aux", bufs=2,
                                 name=f"wu{i}")
                nc.tensor.matmul(wu_ps[:], wu_a[:], wu_b[:],
                                 start=True, stop=True)

            # Input DMAs on the two hardware DGE queues (sync + scalar),
            # alternated per chunk, in consumption order.  Each DMA costs
            # ~0.7us of issue time on its engine (descriptor gen per row),
            # so transfers are as wide as possible.
            # DMA-issue instructions BLOCK the issuing engine once the HW
            # queue fills, so the scalar (ACT) queue gets only what it can
            # finish before qkv0's drains need the engine (~19us); the long
            # tail (x1-half, wpT, x2/x3) goes on sync, whose engine has no
            # latency-critical role until the first out-DMAs (~45us).
            nc.scalar.dma_start(bqk_sb[:], bqk_d[:])
            nc.sync.dma_start(mask_sb[:], mask_d[:])
            nc.scalar.dma_start(bv_row[:], bv_d[:])
            nc.gpsimd.partition_broadcast(bv_bc[:], bv_row[:])
            for k in range(KC):
                eng = nc.sync if k % 2 == 0 else nc.scalar
                eng2 = nc.scalar if k % 2 == 0 else nc.sync
                eng.dma_start(wqk_sb[k][:], wqk_d[128 * k:128 * (k + 1), :])
                eng2.dma_start(xT01[0][k][:],
                               xT_d[128 * k:128 * (k + 1), 0:512])
            for k in range(KC):
                nc.scalar.dma_start(wv_sb[k][:],
                                    wv_d[128 * k:128 * (k + 1), :])
            for k in range(KC):
                eng = nc.scalar if k % 2 == 0 else nc.sync
                eng.dma_start(xT01[1][k][:],
                              xT_d[128 * k:128 * (k + 1), 512:1024])
            for k in range(2):
                nc.sync.dma_start(wpT_sb[k][:], wp_d[128 * k:128 * (k + 1), :])
            for k in range(KC):
                nc.sync.dma_start(xT23[k][:],
                                  xT_d[128 * k:128 * (k + 1), 1024:2048])
            # ones columns of V1 (written once; disjoint from the data cols)
            for m in range(4 * NTC):
                vv = V1[m].rearrange("p (h x) -> p h x", h=HPG)
                nc.vector.memset(vv[:, :, HD:HD + 1], 1.0)

            # ---- qkv building blocks ----
            def qkv_q_group(t, jh, sel, drain_on_act=False):
                """One Q-or-K psum group: 8 MMs + bias drain.
                sel 0 -> Q (w cols [0:DG]), sel 1 -> K (w cols [DG:2DG])."""
                dst = QT if sel == 0 else KT

                def thunk():
                    ps = psp.tile([128, 512], f32, tag="aux", bufs=2,
                                  name=f"qk{t}_{sel}_{jh}")
                    for kc in range(KC):
                        nc.tensor.matmul(
                            ps[:],
                            wqk_sb[kc][:, DG * sel + 128 * jh:
                                       DG * sel + 128 * (jh + 1)],
                            x_sl(t, kc),
                            start=(kc == 0), stop=(kc == KC - 1))
                    bcol = bqk_sb[:, 2 * sel + jh:2 * sel + jh + 1]
                    if drain_on_act:
                        nc.scalar.activation(out=dst[t][jh][:], in_=ps[:],
                                             func=Ident, bias=bcol)
                    else:
                        nc.vector.tensor_scalar_add(dst[t][jh][:], ps[:],
                                                    bcol)
                return thunk

            def qkv_v_group(t, mt):
                def thunk():
                    psv = psp.tile([128, 512], f32, tag="aux", bufs=2,
                                   name=f"vps{t}_{mt}")
                    for kc in range(KC):
                        nc.tensor.matmul(
                            psv[:, 0:DG],
                            x_sl(t, kc, 128 * mt, 128 * (mt + 1)),
                            wv_sb[kc][:],
                            start=(kc == 0), stop=(kc == KC - 1))
                    vv = V1[4 * t + mt].rearrange("p (h x) -> p h x", h=HPG)
                    nc.vector.tensor_add(
                        vv[:, :, 0:HD],
                        psv[:, 0:DG].rearrange("p (h x) -> p h x", h=HPG),
                        bv_bc.rearrange("p (h x) -> p h x", h=HPG))
                return thunk

            def qkv_groups(t, drain_on_act=False):
                gs = [qkv_q_group(t, jh, sel, drain_on_act)
                      for sel in range(2) for jh in range(2)]
                gs += [qkv_v_group(t, mt) for mt in range(4)]
                return gs

            # ---- output projection (one half = 4 eh columns) ----
            def proj_half(cq, yns, ehs, tail=False):
                def thunk():
                    for eh in ehs:
                        po = psp.tile([128, 512], f32, tag="aux", bufs=2,
                                      name=f"po_{cq}_{eh}")
                        nc.tensor.matmul(po[:],
                                         wpT_sb[0][:, 128 * eh:128 * (eh + 1)],
                                         yns[0][:], start=True, stop=False)
                        nc.tensor.matmul(po[:],
                                         wpT_sb[1][:, 128 * eh:128 * (eh + 1)],
                                         yns[1][:], start=False, stop=True)
                        ot = otp.tile([128, 512], bf16, tag="ot", bufs=4,
                                      name=f"ot_{cq}_{eh}")
                        if tail and eh % 2 == 1:
                            nc.scalar.activation(out=ot[:], in_=po[:],
                                                 func=Copy)
                        else:
                            nc.vector.tensor_copy(ot[:], po[:])
                        eng = nc.scalar if (tail and eh % 2 == 0) else nc.sync
                        eng.dma_start(
                            oP_d[128 * eh:128 * (eh + 1),
                                 512 * cq:512 * (cq + 1)], ot[:])
                return thunk

            # ---- attention phase with interleaved fillers ----
            def att(cq, fillers_h, end_frac=0.72):
                """Flash attention for q-chunk cq.  fillers_h = (h0, h1):
                per-p-half filler lists, each ceil-paced over the first
                end_frac of that half's kt slots (so the Vector FIFO drains
                before the normalize at the end of each half)."""
                nkt = 4 * (cq + 1)
                win = max(1, int(round(end_frac * nkt)))
                yns = []
                for p in range(2):
                    fl = fillers_h[p]
                    nfl = len(fl)
                    emitted = 0
                    yps = [psp.tile([HD + 1, 512], f32, tag=f"y{X}", bufs=1,
                                    name=f"y_{cq}_{p}_{X}") for X in range(2)]

                    def emit_av(kt, Pt, qs):
                        for X in range(2):
                            h = 2 * p + X
                            nc.tensor.matmul(
                                yps[X][:, qs:512],
                                V1[kt][:, VW * h:VW * h + HD + 1],
                                Pt[:, 512 * X + qs:512 * (X + 1)],
                                start=(kt == 0), stop=(kt == nkt - 1))

                    pend = None   # AV runs one k-tile behind S/exp
                    for kt in range(nkt):
                        qs = max(0, 128 * kt - 512 * cq)
                        qs2 = min(qs, 256)   # keep f32r free dim >= 256
                        S = psp.tile([128, 1024], f32, tag="s", bufs=2,
                                     name=f"s_{cq}_{p}_{kt}")
                        for X in range(2):
                            nc.tensor.matmul(
                                S[:, 512 * X + qs2:512 * (X + 1)],
                                KT[kt // 4][p][64 * X:64 * (X + 1),
                                               128 * (kt % 4):128 * (kt % 4 + 1)],
                                QT[cq][p][64 * X:64 * (X + 1), qs2:512],
                                start=True, stop=True)
                        if pend is not None:
                            emit_av(*pend)
                        Pt = ppool.tile([128, 1024], bf16, tag="p", bufs=4,
                                        name=f"p_{cq}_{p}_{kt}")
                        nc.scalar.activation(
                            out=Pt.rearrange("pp (x q) -> pp x q",
                                             x=2)[:, :, qs:512],
                            in_=S.rearrange("pp (x q) -> pp x q",
                                            x=2)[:, :, qs:512],
                            func=Exp, scale=1.0 / math.sqrt(HD))
                        if kt >= 4 * cq:  # diagonal block: causal mask
                            for X in range(2):
                                nc.gpsimd.tensor_mul(
                                    Pt[:, 512 * X + qs:512 * X + qs + 128],
                                    Pt[:, 512 * X + qs:512 * X + qs + 128],
                                    mask_sb[:])
                        pend = (kt, Pt, qs)
                        if nfl:
                            tgt = min(nfl,
                                      int(math.ceil(nfl * (kt + 1) / win)))
                            while emitted < tgt:
                                fl[emitted]()
                                emitted += 1
                    emit_av(*pend)
                    # normalize: drain psum fast, then recip/broadcast/mul
                    yn = ynp.tile([128, 512], bf16, tag="yn", bufs=4,
                                  name=f"yn_{cq}_{p}")
                    for X in range(2):
                        ycp = npool.tile([HD, 512], bf16, tag="ycp", bufs=4,
                                         name=f"yc_{cq}_{p}_{X}")
                        nc.vector.tensor_copy(ycp[:], yps[X][0:HD, :])
                        r1 = npool.tile([1, 512], f32, tag="r1", bufs=4,
                                        name=f"r1_{cq}_{p}_{X}")
                        nc.vector.tensor_copy(r1[:], yps[X][HD:HD + 1, :])
                        rr = npool.tile([1, 512], f32, tag="rr", bufs=4,
                                        name=f"rr_{cq}_{p}_{X}")
                        nc.vector.reciprocal_approx_fast(out=rr[:], in_=r1[:])
                        rrb = npool.tile([1, 512], bf16, tag="rrb", bufs=4,
                                         name=f"rrb_{cq}_{p}_{X}")
                        nc.vector.tensor_copy(rrb[:], rr[:])
                        bcx = npool.tile([HD, 512], bf16, tag="bc", bufs=4,
                                         name=f"bcx_{cq}_{p}_{X}")
                        nc.gpsimd.partition_broadcast(bcx[:], rrb[:])
                        nc.gpsimd.tensor_mul(
                            yn[64 * X:64 * (X + 1), :], ycp[:], bcx[:])
                    yns.append(yn)
                    while emitted < nfl:
                        fl[emitted]()
                        emitted += 1
                return yns

            # ---- the pipeline ----
            # qkv(0) before att(0); its Q/K drains go on ScalarE (idle then)
            for g in qkv_groups(0, drain_on_act=True):
                g()

            # PHASE 0: att(0) + qkv(1), delayed into the p=1 half (x1 DMA)
            yns0 = att(0, ([], qkv_groups(1)), end_frac=0.8)
            # PHASE 1: att(1) + proj(0) + Q(2)/K(2)  (x2 lands ~mid-phase)
            f1 = ([proj_half(0, yns0, [0, 1, 2, 3]),
                   proj_half(0, yns0, [4, 5, 6, 7])],
                  [qkv_q_group(2, 0, 0), qkv_q_group(2, 1, 0),
                   qkv_q_group(2, 0, 1), qkv_q_group(2, 1, 1)])
            yns1 = att(1, f1)
            # PHASE 2: att(2) + V(2) + Q(3) + proj(1)
            f2 = ([qkv_v_group(2, mt) for mt in range(4)],
                  [qkv_q_group(3, 0, 0), qkv_q_group(3, 1, 0),
                   proj_half(1, yns1, [0, 1, 2, 3]),
                   proj_half(1, yns1, [4, 5, 6, 7])])
            yns2 = att(2, f2, end_frac=0.67)
            # PHASE 3: att(3) + K(3) + V(3) + proj(2)
            f3 = ([qkv_q_group(3, 0, 1), qkv_q_group(3, 1, 1)]
                  + [qkv_v_group(3, mt) for mt in range(4)],
                  [proj_half(2, yns2, [0, 1, 2, 3]),
                   proj_half(2, yns2, [4, 5, 6, 7])])
            yns3 = att(3, f3, end_frac=0.67)
            # tail: proj(3) with drains/DMA split across engines
            proj_half(3, yns3, [0, 1, 2, 3], tail=True)()
            proj_half(3, yns3, [4, 5, 6, 7], tail=True)()

    nc.finalize()
    return nc


def _get_nc():
    if "nc" not in _NC_CACHE:
        _NC_CACHE["nc"] = _build()
    return _NC_CACHE["nc"]


def kernel(x, w_attn, b_attn, w_proj, b_proj):
    from concourse.bass_utils import run_bass_kernel_spmd

    x = np.asarray(x, dtype=np.float32)
    w_attn = np.asarray(w_attn, dtype=np.float32)
    b_attn = np.asarray(b_attn, dtype=np.float32)
    w_proj = np.asarray(w_proj, dtype=np.float32)
    b_proj = np.asarray(b_proj, dtype=np.float32)

    mask = np.triu(np.ones((128, 128), dtype=np.float32)).copy()
    wpT_full = np.ascontiguousarray(w_proj.T)  # [C_in, C_out]

    in_maps = []
    for c in range(N_CORES):
        b, g = divmod(c, G)
        lo = DG * g
        wq = w_attn[lo:lo + DG, :].T                    # [C, DG]
        wk = w_attn[C + lo:C + lo + DG, :].T
        wv = w_attn[2 * C + lo:2 * C + lo + DG, :].T
        wqk = np.concatenate([wq, wk], axis=1)          # [C, 2*DG]
        bqk = np.stack([b_attn[lo:lo + 128],
                        b_attn[lo + 128:lo + 256],
                        b_attn[C + lo:C + lo + 128],
                        b_attn[C + lo + 128:C + lo + 256]], axis=1)  # [128,4]
        in_maps.append({
            "xT": np.ascontiguousarray(x[b].T).astype(ml_dtypes.bfloat16),
            "wqk": np.ascontiguousarray(wqk).astype(ml_dtypes.bfloat16),
            "wv": np.ascontiguousarray(wv).astype(ml_dtypes.bfloat16),
            "bqk": np.ascontiguousarray(bqk.astype(np.float32)),
            "bv": np.ascontiguousarray(
                b_attn[2 * C + lo:2 * C + lo + DG].reshape(1, DG)
                .astype(np.float32)),
            "wpT": np.ascontiguousarray(wpT_full[lo:lo + DG, :]).astype(ml_dtypes.bfloat16),
            "mask": mask.astype(ml_dtypes.bfloat16),
        })

    global _last_in_maps
    _last_in_maps = in_maps

    nc = _get_nc()
    res = run_bass_kernel_spmd(nc, in_maps, list(range(N_CORES)))

    out = np.empty((B, T, C), dtype=np.float32)
    for b in range(B):
        acc = np.zeros((C, T), dtype=np.float32)
        for g in range(G):
            acc += res.results[4 * b + g]["oP"].astype(np.float32)
        out[b] = acc.T + b_proj
    return out


# revision 27
# speedup vs baseline: 1.1634x; 1.1634x over previous
"""Causal self-attention (B=2, T=2048, C=1024, H=16) on 8 trn2 NeuronCores.

Sharding: core c = (b, g) with b = c // 4 (batch), g = c % 4 (head-group of 4
heads = 256 dims).  No collectives: each core computes a PARTIAL output
projection over its own 256 head-dims (o_part^T = W_g^T y_g^T, bf16) and the
host sums the 4 partials per batch during unsharding.

v3 schedule: the attention phases are ScalarE(exp)-bound, so QKV / output
projection matmul groups are emitted as "fillers" interleaved INTO the
attention kt-loops -- the in-order PE queue always has independent ready work
while exp lags.  Fillers are paced away from the p-boundaries so the Vector
FIFO drains before each normalize (whose PSUM reads gate the y-bank handoff
to the next p half).  Input DMAs are merged (w_qkv as one [C,768] tensor,
biases packed) to cut the ~0.7us/instruction descriptor-generation cost, and
alternate between the two hardware DGE queues in consumption order.
"""
import math

import numpy as np
import ml_dtypes

B, T, C, H = 2, 2048, 1024, 16
HD = C // H          # 64 head dim
G = 4                # head-groups (cores per batch)
HPG = H // G         # 4 heads per group
DG = HPG * HD        # 256 dims per group
N_CORES = 8
KC = C // 128        # 8 contraction chunks
NTC = T // 512       # 4 t-chunks (and attention q-chunks)
VW = HD + 2          # V1 per-head stride (64 data + 1 ones + 1 pad)

_NC_CACHE = {}


def _build():
    import concourse.bacc as bacc
    import concourse.mybir as mybir
    import concourse.tile as tile

    f32 = mybir.dt.float32
    f32r = mybir.dt.float32r
    bf16 = mybir.dt.bfloat16
    Exp = mybir.ActivationFunctionType.Exp
    Ident = mybir.ActivationFunctionType.Identity
    Copy = mybir.ActivationFunctionType.Copy

    nc = bacc.Bacc("TRN2", num_devices=N_CORES)

    xT_d = nc.dram_tensor("xT", [C, T], bf16, kind="ExternalInput")
    # merged [wq | wk] columns, pre-transposed: [C, 2*DG]; wv separate
    wqk_d = nc.dram_tensor("wqk", [C, 2 * DG], bf16, kind="ExternalInput")
    wv_d = nc.dram_tensor("wv", [C, DG], bf16, kind="ExternalInput")
    # packed biases: cols = [bq jh0, bq jh1, bk jh0, bk jh1]
    bqk_d = nc.dram_tensor("bqk", [128, 4], f32, kind="ExternalInput")
    bv_d = nc.dram_tensor("bv", [1, DG], f32, kind="ExternalInput")
    # w_proj.T rows [lo:lo+DG] -> partial projection stationary [DG, C]
    wp_d = nc.dram_tensor("wpT", [DG, C], bf16, kind="ExternalInput")
    mask_d = nc.dram_tensor("mask", [128, 128], bf16, kind="ExternalInput")
    oP_d = nc.dram_tensor("oP", [C, T], bf16, kind="ExternalOutput")

    with tile.TileContext(nc) as tc:
        with (
            tc.tile_pool(name="persist", bufs=1) as persist,
            tc.tile_pool(name="xp", bufs=1) as xp,
            tc.tile_pool(name="wp_s", bufs=1) as wp_s,
            tc.tile_pool(name="psp", bufs=1, space="PSUM") as psp,
            tc.tile_pool(name="ppool", bufs=1) as ppool,
            tc.tile_pool(name="npool", bufs=1) as npool,
            tc.tile_pool(name="ynp", bufs=1) as ynp,
            tc.tile_pool(name="otp", bufs=1) as otp,
        ):
            # ---- persistent SBUF ----
            QT = [[persist.tile([128, 512], f32r, name=f"qt{t}_{j}")
                   for j in range(2)] for t in range(NTC)]
            KT = [[persist.tile([128, 512], f32r, name=f"kt{t}_{j}")
                   for j in range(2)] for t in range(NTC)]
            V1 = [persist.tile([128, HPG * VW], bf16, name=f"v{m}")
                  for m in range(4 * NTC)]
            # t=0,1: one tile per (t, k); t=2,3 merged: one [128,1024] per k
            xT01 = [[xp.tile([128, 512], bf16, name=f"x{t}_{k}")
                     for k in range(KC)] for t in range(2)]
            xT23 = [xp.tile([128, 1024], bf16, name=f"x23_{k}")
                    for k in range(KC)]

            def x_sl(t, k, lo=0, hi=512):
                if t < 2:
                    return xT01[t][k][:, lo:hi]
                return xT23[k][:, 512 * (t - 2) + lo:512 * (t - 2) + hi]
            wqk_sb = [wp_s.tile([128, 2 * DG], bf16, name=f"w{k}")
                      for k in range(KC)]
            wv_sb = [wp_s.tile([128, DG], bf16, name=f"wv{k}")
                     for k in range(KC)]
            wpT_sb = [persist.tile([128, C], bf16, name=f"wp_{k}")
                      for k in range(2)]
            mask_sb = persist.tile([128, 128], bf16, name="mask_sb")
            bqk_sb = persist.tile([128, 4], f32, name="bqk_sb")
            bv_row = persist.tile([1, DG], f32, name="bv_row")
            bv_bc = persist.tile([128, DG], f32, name="bv_bc")

            # ---- prologue: warmup + DMA streaming ----
            wu_a = wp_s.tile([128, 128], bf16, name="wu_a")
            wu_b = wp_s.tile([128, 512], bf16, name="wu_b")
            nc.vector.memset(wu_a[:], 0.5)
            nc.vector.memset(wu_b[:], 0.5)
            for i in range(40):
                wu_ps = psp.tile([128, 512], f32, tag="aux", bufs=2,
                                 name=f"wu{i}")
                nc.tensor.matmul(wu_ps[:], wu_a[:], wu_b[:],
                                 start=True, stop=True)

            # Input DMAs on the two hardware DGE queues (sync + scalar),
            # alternated per chunk, in consumption order.  Each DMA costs
            # ~0.7us of issue time on its engine (descriptor gen per row),
            # so transfers are as wide as possible.
            # DMA-issue instructions BLOCK the issuing engine once the HW
            # queue fills, so the scalar (ACT) queue gets only what it can
            # finish before qkv0's drains need the engine (~19us); the long
            # tail (x1-half, wpT, x2/x3) goes on sync, whose engine has no
            # latency-critical role until the first out-DMAs (~45us).
            nc.scalar.dma_start(bqk_sb[:], bqk_d[:])
            nc.sync.dma_start(mask_sb[:], mask_d[:])
            nc.scalar.dma_start(bv_row[:], bv_d[:])
            nc.gpsimd.partition_broadcast(bv_bc[:], bv_row[:])
            # The scalar (ACT) queue only gets DMAs it can finish issuing
            # before qkv0's drains need the engine; everything later-used
            # goes on sync so att-phase exps are never stuck behind issues.
            for k in range(KC):
                eng = nc.sync if k % 2 == 0 else nc.scalar
                eng2 = nc.scalar if k % 2 == 0 else nc.sync
                eng.dma_start(wqk_sb[k][:], wqk_d[128 * k:128 * (k + 1), :])
                eng2.dma_start(xT01[0][k][:],
                               xT_d[128 * k:128 * (k + 1), 0:512])
            for k in range(KC):
                eng = nc.scalar if k % 2 == 0 else nc.sync
                eng.dma_start(wv_sb[k][:], wv_d[128 * k:128 * (k + 1), :])
            for k in range(KC):
                nc.sync.dma_start(xT01[1][k][:],
                                  xT_d[128 * k:128 * (k + 1), 512:1024])
            for k in range(2):
                nc.sync.dma_start(wpT_sb[k][:], wp_d[128 * k:128 * (k + 1), :])
            for k in range(KC):
                nc.sync.dma_start(xT23[k][:],
                                  xT_d[128 * k:128 * (k + 1), 1024:2048])
            # ones columns of V1 (written once; disjoint from the data cols)
            for m in range(4 * NTC):
                vv = V1[m].rearrange("p (h x) -> p h x", h=HPG)
                nc.vector.memset(vv[:, :, HD:HD + 1], 1.0)

            # ---- qkv building blocks ----
            def qkv_q_group(t, jh, sel, drain_on_act=False):
                """One Q-or-K psum group: 8 MMs + bias drain.
                sel 0 -> Q (w cols [0:DG]), sel 1 -> K (w cols [DG:2DG])."""
                dst = QT if sel == 0 else KT

                def thunk():
                    ps = psp.tile([128, 512], f32, tag="aux", bufs=2,
                                  name=f"qk{t}_{sel}_{jh}")
                    for kc in range(KC):
                        nc.tensor.matmul(
                            ps[:],
                            wqk_sb[kc][:, DG * sel + 128 * jh:
                                       DG * sel + 128 * (jh + 1)],
                            x_sl(t, kc),
                            start=(kc == 0), stop=(kc == KC - 1))
                    bcol = bqk_sb[:, 2 * sel + jh:2 * sel + jh + 1]
                    if drain_on_act:
                        nc.scalar.activation(out=dst[t][jh][:], in_=ps[:],
                                             func=Ident, bias=bcol)
                    else:
                        nc.vector.tensor_scalar_add(dst[t][jh][:], ps[:],
                                                    bcol)
                return thunk

            def qkv_v_group(t, mt):
                def thunk():
                    psv = psp.tile([128, 512], f32, tag="aux", bufs=2,
                                   name=f"vps{t}_{mt}")
                    for kc in range(KC):
                        nc.tensor.matmul(
                            psv[:, 0:DG],
                            x_sl(t, kc, 128 * mt, 128 * (mt + 1)),
                            wv_sb[kc][:],
                            start=(kc == 0), stop=(kc == KC - 1))
                    vv = V1[4 * t + mt].rearrange("p (h x) -> p h x", h=HPG)
                    nc.vector.tensor_add(
                        vv[:, :, 0:HD],
                        psv[:, 0:DG].rearrange("p (h x) -> p h x", h=HPG),
                        bv_bc.rearrange("p (h x) -> p h x", h=HPG))
                return thunk

            def qkv_groups(t, drain_on_act=False):
                gs = [qkv_q_group(t, jh, sel, drain_on_act)
                      for sel in range(2) for jh in range(2)]
                gs += [qkv_v_group(t, mt) for mt in range(4)]
                return gs

            # ---- output projection (one half = 4 eh columns) ----
            def proj_half(cq, yns, ehs, tail=False):
                def thunk():
                    for eh in ehs:
                        po = psp.tile([128, 512], f32, tag="aux", bufs=2,
                                      name=f"po_{cq}_{eh}")
                        nc.tensor.matmul(po[:],
                                         wpT_sb[0][:, 128 * eh:128 * (eh + 1)],
                                         yns[0][:], start=True, stop=False)
                        nc.tensor.matmul(po[:],
                                         wpT_sb[1][:, 128 * eh:128 * (eh + 1)],
                                         yns[1][:], start=False, stop=True)
                        ot = otp.tile([128, 512], bf16, tag="ot", bufs=4,
                                      name=f"ot_{cq}_{eh}")
                        if tail and eh % 2 == 1:
                            nc.scalar.activation(out=ot[:], in_=po[:],
                                                 func=Copy)
                        else:
                            nc.vector.tensor_copy(ot[:], po[:])
                        eng = nc.scalar if (tail and eh % 2 == 0) else nc.sync
                        eng.dma_start(
                            oP_d[128 * eh:128 * (eh + 1),
                                 512 * cq:512 * (cq + 1)], ot[:])
                return thunk

            # ---- attention phase with interleaved fillers ----
            def att(cq, fillers_h, end_frac=0.72):
                """Flash attention for q-chunk cq.  fillers_h = (h0, h1):
                per-p-half filler lists, each ceil-paced over the first
                end_frac of that half's kt slots (so the Vector FIFO drains
                before the normalize at the end of each half)."""
                nkt = 4 * (cq + 1)
                win = max(1, int(round(end_frac * nkt)))
                yns = []
                for p in range(2):
                    fl = fillers_h[p]
                    nfl = len(fl)
                    emitted = 0
                    yps = [psp.tile([HD + 1, 512], f32, tag=f"y{X}", bufs=1,
                                    name=f"y_{cq}_{p}_{X}") for X in range(2)]

                    def emit_av(kt, Pt, qs):
                        for X in range(2):
                            h = 2 * p + X
                            nc.tensor.matmul(
                                yps[X][:, qs:512],
                                V1[kt][:, VW * h:VW * h + HD + 1],
                                Pt[:, 512 * X + qs:512 * (X + 1)],
                                start=(kt == 0), stop=(kt == nkt - 1))

                    pend = None   # AV runs one k-tile behind S/exp
                    for kt in range(nkt):
                        qs = max(0, 128 * kt - 512 * cq)
                        qs2 = min(qs, 256)   # keep f32r free dim >= 256
                        S = psp.tile([128, 1024], f32, tag="s", bufs=2,
                                     name=f"s_{cq}_{p}_{kt}")
                        for X in range(2):
                            nc.tensor.matmul(
                                S[:, 512 * X + qs2:512 * (X + 1)],
                                KT[kt // 4][p][64 * X:64 * (X + 1),
                                               128 * (kt % 4):128 * (kt % 4 + 1)],
                                QT[cq][p][64 * X:64 * (X + 1), qs2:512],
                                start=True, stop=True)
                        if pend is not None:
                            emit_av(*pend)
                        Pt = ppool.tile([128, 1024], bf16, tag="p", bufs=4,
                                        name=f"p_{cq}_{p}_{kt}")
                        nc.scalar.activation(
                            out=Pt.rearrange("pp (x q) -> pp x q",
                                             x=2)[:, :, qs:512],
                            in_=S.rearrange("pp (x q) -> pp x q",
                                            x=2)[:, :, qs:512],
                            func=Exp, scale=1.0 / math.sqrt(HD))
                        if kt >= 4 * cq:  # diagonal block: causal mask
                            for X in range(2):
                                nc.gpsimd.tensor_mul(
                                    Pt[:, 512 * X + qs:512 * X + qs + 128],
                                    Pt[:, 512 * X + qs:512 * X + qs + 128],
                                    mask_sb[:])
                        pend = (kt, Pt, qs)
                        if nfl:
                            tgt = min(nfl,
                                      int(math.ceil(nfl * (kt + 1) / win)))
                            while emitted < tgt:
                                fl[emitted]()
                                emitted += 1
                    emit_av(*pend)
                    # normalize: drain psum fast, then recip/broadcast/mul
                    yn = ynp.tile([128, 512], bf16, tag="yn", bufs=4,
                                  name=f"yn_{cq}_{p}")
                    for X in range(2):
                        ycp = npool.tile([HD, 512], bf16, tag="ycp", bufs=4,
                                         name=f"yc_{cq}_{p}_{X}")
                        nc.vector.tensor_copy(ycp[:], yps[X][0:HD, :])
                        r1 = npool.tile([1, 512], f32, tag="r1", bufs=4,
                                        name=f"r1_{cq}_{p}_{X}")
                        nc.vector.tensor_copy(r1[:], yps[X][HD:HD + 1, :])
                        rr = npool.tile([1, 512], f32, tag="rr", bufs=4,
                                        name=f"rr_{cq}_{p}_{X}")
                        nc.vector.reciprocal_approx_fast(out=rr[:], in_=r1[:])
                        rrb = npool.tile([1, 512], bf16, tag="rrb", bufs=4,
                                         name=f"rrb_{cq}_{p}_{X}")
                        nc.vector.tensor_copy(rrb[:], rr[:])
                        bcx = npool.tile([HD, 512], bf16, tag="bc", bufs=4,
                                         name=f"bcx_{cq}_{p}_{X}")
                        nc.gpsimd.partition_broadcast(bcx[:], rrb[:])
                        nc.gpsimd.tensor_mul(
                            yn[64 * X:64 * (X + 1), :], ycp[:], bcx[:])
                    yns.append(yn)
                    while emitted < nfl:
                        fl[emitted]()
                        emitted += 1
                return yns

            # ---- the pipeline ----
            # qkv(0) before att(0); its Q/K drains go on ScalarE (idle then)
            for g in qkv_groups(0, drain_on_act=True):
                g()

            # PHASE 0: att(0) + qkv(1), delayed into the p=1 half (x1 DMA)
            yns0 = att(0, ([], qkv_groups(1)), end_frac=0.8)
            # PHASE 1: att(1) + proj(0) + Q(2)/K(2)  (x2 lands ~mid-phase)
            f1 = ([proj_half(0, yns0, [0, 1, 2, 3]),
                   proj_half(0, yns0, [4, 5, 6, 7])],
                  [qkv_q_group(2, 0, 0), qkv_q_group(2, 1, 0),
                   qkv_q_group(2, 0, 1), qkv_q_group(2, 1, 1)])
            yns1 = att(1, f1)
            # PHASE 2: att(2) + V(2) + Q(3) + proj(1)
            f2 = ([qkv_v_group(2, mt) for mt in range(4)],
                  [qkv_q_group(3, 0, 0), qkv_q_group(3, 1, 0),
                   proj_half(1, yns1, [0, 1, 2, 3]),
                   proj_half(1, yns1, [4, 5, 6, 7])])
            yns2 = att(2, f2, end_frac=0.67)
            # PHASE 3: att(3) + K(3) + V(3) + proj(2)
            f3 = ([qkv_q_group(3, 0, 1), qkv_q_group(3, 1, 1)]
                  + [qkv_v_group(3, mt) for mt in range(4)],
                  [proj_half(2, yns2, [0, 1, 2, 3]),
                   proj_half(2, yns2, [4, 5, 6, 7])])
            yns3 = att(3, f3, end_frac=0.67)
            # tail: proj(3) with drains/DMA split across engines
            proj_half(3, yns3, [0, 1, 2, 3], tail=True)()
            proj_half(3, yns3, [4, 5, 6, 7], tail=True)()

    nc.finalize()
    return nc


def _get_nc():
    if "nc" not in _NC_CACHE:
        _NC_CACHE["nc"] = _build()
    return _NC_CACHE["nc"]


def kernel(x, w_attn, b_attn, w_proj, b_proj):
    from concourse.bass_utils import run_bass_kernel_spmd

    x = np.asarray(x, dtype=np.float32)
    w_attn = np.asarray(w_attn, dtype=np.float32)
    b_attn = np.asarray(b_attn, dtype=np.float32)
    w_proj = np.asarray(w_proj, dtype=np.float32)
    b_proj = np.asarray(b_proj, dtype=np.float32)

    mask = np.triu(np.ones((128, 128), dtype=np.float32)).copy()
    wpT_full = np.ascontiguousarray(w_proj.T)  # [C_in, C_out]

    in_maps = []
    for c in range(N_CORES):
        b, g = divmod(c, G)
        lo = DG * g
        wq = w_attn[lo:lo + DG, :].T                    # [C, DG]
        wk = w_attn[C + lo:C + lo + DG, :].T
        wv = w_attn[2 * C + lo:2 * C + lo + DG, :].T
        wqk = np.concatenate([wq, wk], axis=1)          # [C, 2*DG]
        bqk = np.stack([b_attn[lo:lo + 128],
                        b_attn[lo + 128:lo + 256],
                        b_attn[C + lo:C + lo + 128],
                        b_attn[C + lo + 128:C + lo + 256]], axis=1)  # [128,4]
        in_maps.append({
            "xT": np.ascontiguousarray(x[b].T).astype(ml_dtypes.bfloat16),
            "wqk": np.ascontiguousarray(wqk).astype(ml_dtypes.bfloat16),
            "wv": np.ascontiguousarray(wv).astype(ml_dtypes.bfloat16),
            "bqk": np.ascontiguousarray(bqk.astype(np.float32)),
            "bv": np.ascontiguousarray(
                b_attn[2 * C + lo:2 * C + lo + DG].reshape(1, DG)
                .astype(np.float32)),
            "wpT": np.ascontiguousarray(wpT_full[lo:lo + DG, :]).astype(ml_dtypes.bfloat16),
            "mask": mask.astype(ml_dtypes.bfloat16),
        })

    global _last_in_maps
    _last_in_maps = in_maps

    nc = _get_nc()
    res = run_bass_kernel_spmd(nc, in_maps, list(range(N_CORES)))

    out = np.empty((B, T, C), dtype=np.float32)
    for b in range(B):
        acc = np.zeros((C, T), dtype=np.float32)
        for g in range(G):
            acc += res.results[4 * b + g]["oP"].astype(np.float32)
        out[b] = acc.T + b_proj
    return out


# revision 29
# speedup vs baseline: 1.2343x; 1.0609x over previous
"""Causal self-attention (B=2, T=2048, C=1024, H=16) on 8 trn2 NeuronCores.

Sharding: core c = (b, g) with b = c // 4 (batch), g = c % 4 (head-group of 4
heads = 256 dims).  No collectives: each core computes a PARTIAL output
projection over its own 256 head-dims (o_part^T = W_g^T y_g^T, bf16) and the
host sums the 4 partials per batch during unsharding.

v3 schedule: the attention phases are ScalarE(exp)-bound, so QKV / output
projection matmul groups are emitted as "fillers" interleaved INTO the
attention kt-loops -- the in-order PE queue always has independent ready work
while exp lags.  Fillers are paced away from the p-boundaries so the Vector
FIFO drains before each normalize (whose PSUM reads gate the y-bank handoff
to the next p half).  Input DMAs are merged (w_qkv as one [C,768] tensor,
biases packed) to cut the ~0.7us/instruction descriptor-generation cost, and
alternate between the two hardware DGE queues in consumption order.
"""
import math

import numpy as np
import ml_dtypes

B, T, C, H = 2, 2048, 1024, 16
HD = C // H          # 64 head dim
G = 4                # head-groups (cores per batch)
HPG = H // G         # 4 heads per group
DG = HPG * HD        # 256 dims per group
N_CORES = 8
KC = C // 128        # 8 contraction chunks
NTC = T // 512       # 4 t-chunks (and attention q-chunks)
VW = HD + 2          # V1 per-head stride (64 data + 1 ones + 1 pad)

_NC_CACHE = {}


def _build():
    import concourse.bacc as bacc
    import concourse.mybir as mybir
    import concourse.tile as tile

    f32 = mybir.dt.float32
    f32r = mybir.dt.float32r
    bf16 = mybir.dt.bfloat16
    Exp = mybir.ActivationFunctionType.Exp
    Ident = mybir.ActivationFunctionType.Identity
    Copy = mybir.ActivationFunctionType.Copy

    nc = bacc.Bacc("TRN2", num_devices=N_CORES)

    xT_d = nc.dram_tensor("xT", [C, T], bf16, kind="ExternalInput")
    # merged [wq | wk] columns, pre-transposed: [C, 2*DG]; wv separate
    wqk_d = nc.dram_tensor("wqk", [C, 2 * DG], bf16, kind="ExternalInput")
    wv_d = nc.dram_tensor("wv", [C, DG], bf16, kind="ExternalInput")
    # packed biases: cols = [bq jh0, bq jh1, bk jh0, bk jh1]
    bqk_d = nc.dram_tensor("bqk", [128, 4], f32, kind="ExternalInput")
    bv_d = nc.dram_tensor("bv", [1, DG], f32, kind="ExternalInput")
    # w_proj.T rows [lo:lo+DG] -> partial projection stationary [DG, C]
    wp_d = nc.dram_tensor("wpT", [DG, C], bf16, kind="ExternalInput")
    mask_d = nc.dram_tensor("mask", [128, 128], bf16, kind="ExternalInput")
    oP_d = nc.dram_tensor("oP", [C, T], bf16, kind="ExternalOutput")

    with tile.TileContext(nc) as tc:
        with (
            tc.tile_pool(name="persist", bufs=1) as persist,
            tc.tile_pool(name="xp", bufs=1) as xp,
            tc.tile_pool(name="wp_s", bufs=1) as wp_s,
            tc.tile_pool(name="psp", bufs=1, space="PSUM") as psp,
            tc.tile_pool(name="ppool", bufs=1) as ppool,
            tc.tile_pool(name="npool", bufs=1) as npool,
            tc.tile_pool(name="ynp", bufs=1) as ynp,
            tc.tile_pool(name="otp", bufs=1) as otp,
        ):
            # ---- persistent SBUF ----
            QT = [[persist.tile([128, 512], f32r, name=f"qt{t}_{j}")
                   for j in range(2)] for t in range(NTC)]
            KT = [[persist.tile([128, 512], f32r, name=f"kt{t}_{j}")
                   for j in range(2)] for t in range(NTC)]
            V1 = [persist.tile([128, HPG * VW], bf16, name=f"v{m}")
                  for m in range(4 * NTC)]
            # t=0,1: one tile per (t, k); t=2,3 merged: one [128,1024] per k
            xT01 = [[xp.tile([128, 512], bf16, name=f"x{t}_{k}")
                     for k in range(KC)] for t in range(2)]
            xT23 = [xp.tile([128, 1024], bf16, name=f"x23_{k}")
                    for k in range(KC)]

            def x_sl(t, k, lo=0, hi=512):
                if t < 2:
                    return xT01[t][k][:, lo:hi]
                return xT23[k][:, 512 * (t - 2) + lo:512 * (t - 2) + hi]
            wqk_sb = [wp_s.tile([128, 2 * DG], bf16, name=f"w{k}")
                      for k in range(KC)]
            wv_sb = [wp_s.tile([128, DG], bf16, name=f"wv{k}")
                     for k in range(KC)]
            wpT_sb = [persist.tile([128, C], bf16, name=f"wp_{k}")
                      for k in range(2)]
            mask_sb = persist.tile([128, 128], bf16, name="mask_sb")
            bqk_sb = persist.tile([128, 4], f32, name="bqk_sb")
            bv_row = persist.tile([1, DG], f32, name="bv_row")
            bv_bc = persist.tile([128, DG], f32, name="bv_bc")

            # ---- prologue: warmup + DMA streaming ----
            wu_a = wp_s.tile([128, 128], bf16, name="wu_a")
            wu_b = wp_s.tile([128, 512], bf16, name="wu_b")
            nc.vector.memset(wu_a[:], 0.5)
            nc.vector.memset(wu_b[:], 0.5)
            for i in range(40):
                wu_ps = psp.tile([128, 512], f32, tag="aux", bufs=2,
                                 name=f"wu{i}")
                nc.tensor.matmul(wu_ps[:], wu_a[:], wu_b[:],
                                 start=True, stop=True)

            # Input DMAs on the two hardware DGE queues (sync + scalar),
            # alternated per chunk, in consumption order.  Each DMA costs
            # ~0.7us of issue time on its engine (descriptor gen per row),
            # so transfers are as wide as possible.
            # DMA-issue instructions BLOCK the issuing engine once the HW
            # queue fills, so the scalar (ACT) queue gets only what it can
            # finish before qkv0's drains need the engine (~19us); the long
            # tail (x1-half, wpT, x2/x3) goes on sync, whose engine has no
            # latency-critical role until the first out-DMAs (~45us).
            nc.scalar.dma_start(bqk_sb[:], bqk_d[:])
            nc.sync.dma_start(mask_sb[:], mask_d[:])
            nc.scalar.dma_start(bv_row[:], bv_d[:])
            nc.gpsimd.partition_broadcast(bv_bc[:], bv_row[:])
            # The scalar (ACT) queue only gets DMAs it can finish issuing
            # before qkv0's drains need the engine; everything later-used
            # goes on sync so att-phase exps are never stuck behind issues.
            for k in range(KC):
                eng = nc.sync if k % 2 == 0 else nc.scalar
                eng2 = nc.scalar if k % 2 == 0 else nc.sync
                eng.dma_start(wqk_sb[k][:], wqk_d[128 * k:128 * (k + 1), :])
                eng2.dma_start(xT01[0][k][:],
                               xT_d[128 * k:128 * (k + 1), 0:512])
            for k in range(KC):
                eng = nc.scalar if k % 2 == 0 else nc.sync
                eng.dma_start(wv_sb[k][:], wv_d[128 * k:128 * (k + 1), :])
            for k in range(KC):
                nc.sync.dma_start(xT01[1][k][:],
                                  xT_d[128 * k:128 * (k + 1), 512:1024])
            for k in range(2):
                nc.sync.dma_start(wpT_sb[k][:], wp_d[128 * k:128 * (k + 1), :])
            for k in range(KC):
                nc.sync.dma_start(xT23[k][:],
                                  xT_d[128 * k:128 * (k + 1), 1024:2048])
            # ones columns of V1 (written once; disjoint from the data cols)
            for m in range(4 * NTC):
                vv = V1[m].rearrange("p (h x) -> p h x", h=HPG)
                nc.vector.memset(vv[:, :, HD:HD + 1], 1.0)

            # ---- qkv building blocks ----
            def qkv_q_group(t, jh, sel, drain_on_act=False):
                """One Q-or-K psum group: 8 MMs + bias drain.
                sel 0 -> Q (w cols [0:DG]), sel 1 -> K (w cols [DG:2DG])."""
                dst = QT if sel == 0 else KT

                def thunk():
                    ps = psp.tile([128, 512], f32, tag="aux", bufs=2,
                                  name=f"qk{t}_{sel}_{jh}")
                    for kc in range(KC):
                        nc.tensor.matmul(
                            ps[:],
                            wqk_sb[kc][:, DG * sel + 128 * jh:
                                       DG * sel + 128 * (jh + 1)],
                            x_sl(t, kc),
                            start=(kc == 0), stop=(kc == KC - 1))
                    bcol = bqk_sb[:, 2 * sel + jh:2 * sel + jh + 1]
                    if drain_on_act:
                        nc.scalar.activation(out=dst[t][jh][:], in_=ps[:],
                                             func=Ident, bias=bcol)
                    else:
                        nc.vector.tensor_scalar_add(dst[t][jh][:], ps[:],
                                                    bcol)
                return thunk

            def qkv_v_group(t, mt):
                def thunk():
                    psv = psp.tile([128, 512], f32, tag="aux", bufs=2,
                                   name=f"vps{t}_{mt}")
                    for kc in range(KC):
                        nc.tensor.matmul(
                            psv[:, 0:DG],
                            x_sl(t, kc, 128 * mt, 128 * (mt + 1)),
                            wv_sb[kc][:],
                            start=(kc == 0), stop=(kc == KC - 1))
                    vv = V1[4 * t + mt].rearrange("p (h x) -> p h x", h=HPG)
                    nc.vector.tensor_add(
                        vv[:, :, 0:HD],
                        psv[:, 0:DG].rearrange("p (h x) -> p h x", h=HPG),
                        bv_bc.rearrange("p (h x) -> p h x", h=HPG))
                return thunk

            def qkv_groups(t, drain_on_act=False):
                gs = [qkv_q_group(t, jh, sel, drain_on_act)
                      for sel in range(2) for jh in range(2)]
                gs += [qkv_v_group(t, mt) for mt in range(4)]
                return gs

            # ---- output projection (one half = 4 eh columns) ----
            def proj_half(cq, yns, ehs, tail=False):
                def thunk():
                    for eh in ehs:
                        po = psp.tile([128, 512], f32, tag="aux", bufs=2,
                                      name=f"po_{cq}_{eh}")
                        nc.tensor.matmul(po[:],
                                         wpT_sb[0][:, 128 * eh:128 * (eh + 1)],
                                         yns[0][:], start=True, stop=False)
                        nc.tensor.matmul(po[:],
                                         wpT_sb[1][:, 128 * eh:128 * (eh + 1)],
                                         yns[1][:], start=False, stop=True)
                        ot = otp.tile([128, 512], bf16, tag="ot", bufs=4,
                                      name=f"ot_{cq}_{eh}")
                        if tail and eh % 2 == 1:
                            nc.scalar.activation(out=ot[:], in_=po[:],
                                                 func=Copy)
                        else:
                            nc.vector.tensor_copy(ot[:], po[:])
                        eng = nc.scalar if (tail and eh % 2 == 0) else nc.sync
                        eng.dma_start(
                            oP_d[128 * eh:128 * (eh + 1),
                                 512 * cq:512 * (cq + 1)], ot[:])
                return thunk

            # ---- attention phase with interleaved fillers ----
            def att(cq, fillers_h, end_frac=0.72):
                """Flash attention for q-chunk cq.  fillers_h = (h0, h1):
                per-p-half filler lists, each ceil-paced over the first
                end_frac of that half's kt slots (so the Vector FIFO drains
                before the normalize at the end of each half)."""
                nkt = 4 * (cq + 1)
                win = max(1, int(round(end_frac * nkt)))
                yns = []
                for p in range(2):
                    fl = fillers_h[p]
                    nfl = len(fl)
                    emitted = 0
                    yps = [psp.tile([HD + 1, 512], f32, tag=f"y{X}", bufs=1,
                                    name=f"y_{cq}_{p}_{X}") for X in range(2)]

                    def emit_av(kt, Pt, qs):
                        for X in range(2):
                            h = 2 * p + X
                            nc.tensor.matmul(
                                yps[X][:, qs:512],
                                V1[kt][:, VW * h:VW * h + HD + 1],
                                Pt[:, 512 * X + qs:512 * (X + 1)],
                                start=(kt == 0), stop=(kt == nkt - 1))

                    pend = None   # AV runs one k-tile behind S/exp
                    for kt in range(nkt):
                        qs = max(0, 128 * kt - 512 * cq)
                        qs2 = min(qs, 256)   # keep f32r free dim >= 256
                        S = psp.tile([128, 1024], f32, tag="s", bufs=2,
                                     name=f"s_{cq}_{p}_{kt}")
                        for X in range(2):
                            nc.tensor.matmul(
                                S[:, 512 * X + qs2:512 * (X + 1)],
                                KT[kt // 4][p][64 * X:64 * (X + 1),
                                               128 * (kt % 4):128 * (kt % 4 + 1)],
                                QT[cq][p][64 * X:64 * (X + 1), qs2:512],
                                start=True, stop=True)
                        if pend is not None:
                            emit_av(*pend)
                        Pt = ppool.tile([128, 1024], bf16, tag="p", bufs=6,
                                        name=f"p_{cq}_{p}_{kt}")
                        nc.scalar.activation(
                            out=Pt.rearrange("pp (x q) -> pp x q",
                                             x=2)[:, :, qs:512],
                            in_=S.rearrange("pp (x q) -> pp x q",
                                            x=2)[:, :, qs:512],
                            func=Exp, scale=1.0 / math.sqrt(HD))
                        if kt >= 4 * cq:  # diagonal block: causal mask
                            for X in range(2):
                                nc.gpsimd.tensor_mul(
                                    Pt[:, 512 * X + qs:512 * X + qs + 128],
                                    Pt[:, 512 * X + qs:512 * X + qs + 128],
                                    mask_sb[:])
                        pend = (kt, Pt, qs)
                        if nfl:
                            tgt = min(nfl,
                                      int(math.ceil(nfl * (kt + 1) / win)))
                            while emitted < tgt:
                                fl[emitted]()
                                emitted += 1
                    emit_av(*pend)
                    # normalize: drain psum fast, then recip/broadcast/mul.
                    # High priority so the scheduler doesn't bury these
                    # latency-critical ops behind filler drains in the
                    # engine FIFOs (the y-bank handoff to the next p half
                    # gates the whole attention pipeline).
                    yn = ynp.tile([128, 512], bf16, tag="yn", bufs=4,
                                  name=f"yn_{cq}_{p}")
                    with tc.high_priority(offset=400):
                        for X in range(2):
                            ycp = npool.tile([HD, 512], bf16, tag="ycp",
                                             bufs=4, name=f"yc_{cq}_{p}_{X}")
                            nc.vector.tensor_copy(ycp[:], yps[X][0:HD, :])
                            r1 = npool.tile([1, 512], f32, tag="r1", bufs=4,
                                            name=f"r1_{cq}_{p}_{X}")
                            nc.vector.tensor_copy(r1[:], yps[X][HD:HD + 1, :])
                            rr = npool.tile([1, 512], f32, tag="rr", bufs=4,
                                            name=f"rr_{cq}_{p}_{X}")
                            nc.vector.reciprocal_approx_fast(out=rr[:],
                                                             in_=r1[:])
                            rrb = npool.tile([1, 512], bf16, tag="rrb",
                                             bufs=4, name=f"rrb_{cq}_{p}_{X}")
                            nc.vector.tensor_copy(rrb[:], rr[:])
                            bcx = npool.tile([HD, 512], bf16, tag="bc",
                                             bufs=4, name=f"bcx_{cq}_{p}_{X}")
                            nc.gpsimd.partition_broadcast(bcx[:], rrb[:])
                            if X == 0:
                                nc.vector.tensor_mul(
                                    yn[0:HD, :], ycp[:], bcx[:])
                            else:
                                nc.gpsimd.tensor_mul(
                                    yn[HD:2 * HD, :], ycp[:], bcx[:])
                    yns.append(yn)
                    while emitted < nfl:
                        fl[emitted]()
                        emitted += 1
                return yns

            # ---- the pipeline ----
            # qkv(0) before att(0); its Q/K drains go on ScalarE (idle then)
            for g in qkv_groups(0, drain_on_act=True):
                g()

            # PHASE 0: att(0) + qkv(1), delayed into the p=1 half (x1 DMA)
            yns0 = att(0, ([], qkv_groups(1)), end_frac=0.8)
            # PHASE 1: att(1) + proj(0) + Q(2)/K(2)  (x2 lands ~mid-phase)
            f1 = ([proj_half(0, yns0, [0, 1, 2, 3]),
                   proj_half(0, yns0, [4, 5, 6, 7])],
                  [qkv_q_group(2, 0, 0), qkv_q_group(2, 1, 0),
                   qkv_q_group(2, 0, 1), qkv_q_group(2, 1, 1)])
            yns1 = att(1, f1)
            # PHASE 2: att(2) + V(2) + Q(3) + proj(1)
            f2 = ([qkv_v_group(2, mt) for mt in range(4)],
                  [qkv_q_group(3, 0, 0), qkv_q_group(3, 1, 0),
                   proj_half(1, yns1, [0, 1, 2, 3]),
                   proj_half(1, yns1, [4, 5, 6, 7])])
            yns2 = att(2, f2, end_frac=0.67)
            # PHASE 3: att(3) + K(3) + V(3) + proj(2)
            f3 = ([qkv_q_group(3, 0, 1), qkv_q_group(3, 1, 1)]
                  + [qkv_v_group(3, mt) for mt in range(4)],
                  [proj_half(2, yns2, [0, 1, 2, 3]),
                   proj_half(2, yns2, [4, 5, 6, 7])])
            yns3 = att(3, f3, end_frac=0.67)
            # tail: proj(3) with drains/DMA split across engines
            proj_half(3, yns3, [0, 1, 2, 3], tail=True)()
            proj_half(3, yns3, [4, 5, 6, 7], tail=True)()

    nc.finalize()
    return nc


def _get_nc():
    if "nc" not in _NC_CACHE:
        _NC_CACHE["nc"] = _build()
    return _NC_CACHE["nc"]


def kernel(x, w_attn, b_attn, w_proj, b_proj):
    from concourse.bass_utils import run_bass_kernel_spmd

    x = np.asarray(x, dtype=np.float32)
    w_attn = np.asarray(w_attn, dtype=np.float32)
    b_attn = np.asarray(b_attn, dtype=np.float32)
    w_proj = np.asarray(w_proj, dtype=np.float32)
    b_proj = np.asarray(b_proj, dtype=np.float32)

    mask = np.triu(np.ones((128, 128), dtype=np.float32)).copy()
    wpT_full = np.ascontiguousarray(w_proj.T)  # [C_in, C_out]

    in_maps = []
    for c in range(N_CORES):
        b, g = divmod(c, G)
        lo = DG * g
        wq = w_attn[lo:lo + DG, :].T                    # [C, DG]
        wk = w_attn[C + lo:C + lo + DG, :].T
        wv = w_attn[2 * C + lo:2 * C + lo + DG, :].T
        wqk = np.concatenate([wq, wk], axis=1)          # [C, 2*DG]
        bqk = np.stack([b_attn[lo:lo + 128],
                        b_attn[lo + 128:lo + 256],
                        b_attn[C + lo:C + lo + 128],
                        b_attn[C + lo + 128:C + lo + 256]], axis=1)  # [128,4]
        in_maps.append({
            "xT": np.ascontiguousarray(x[b].T).astype(ml_dtypes.bfloat16),
            "wqk": np.ascontiguousarray(wqk).astype(ml_dtypes.bfloat16),
            "wv": np.ascontiguousarray(wv).astype(ml_dtypes.bfloat16),
            "bqk": np.ascontiguousarray(bqk.astype(np.float32)),
            "bv": np.ascontiguousarray(
                b_attn[2 * C + lo:2 * C + lo + DG].reshape(1, DG)
                .astype(np.float32)),
            "wpT": np.ascontiguousarray(wpT_full[lo:lo + DG, :]).astype(ml_dtypes.bfloat16),
            "mask": mask.astype(ml_dtypes.bfloat16),
        })

    global _last_in_maps
    _last_in_maps = in_maps

    nc = _get_nc()
    res = run_bass_kernel_spmd(nc, in_maps, list(range(N_CORES)))

    out = np.empty((B, T, C), dtype=np.float32)
    for b in range(B):
        acc = np.zeros((C, T), dtype=np.float32)
        for g in range(G):
            acc += res.results[4 * b + g]["oP"].astype(np.float32)
        out[b] = acc.T + b_proj
    return out


# revision 33
# speedup vs baseline: 1.6962x; 1.3742x over previous
"""Causal self-attention (B=2, T=2048, C=1024, H=16) on 8 trn2 NeuronCores.

Sharding: core c = (b, g) with b = c // 4 (batch), g = c % 4 (head-group of 4
heads = 256 dims).  No collectives: each core computes a PARTIAL output
projection over its own 256 head-dims (o_part^T = W_g^T y_g^T, bf16) and the
host sums the 4 partials per batch during unsharding.

v3 schedule: the attention phases are ScalarE(exp)-bound, so QKV / output
projection matmul groups are emitted as "fillers" interleaved INTO the
attention kt-loops -- the in-order PE queue always has independent ready work
while exp lags.  Fillers are paced away from the p-boundaries so the Vector
FIFO drains before each normalize (whose PSUM reads gate the y-bank handoff
to the next p half).  Input DMAs are merged (w_qkv as one [C,768] tensor,
biases packed) to cut the ~0.7us/instruction descriptor-generation cost, and
alternate between the two hardware DGE queues in consumption order.
"""
import math

import numpy as np
import ml_dtypes

B, T, C, H = 2, 2048, 1024, 16
HD = C // H          # 64 head dim
G = 4                # head-groups (cores per batch)
HPG = H // G         # 4 heads per group
DG = HPG * HD        # 256 dims per group
N_CORES = 8
KC = C // 128        # 8 contraction chunks
NTC = T // 512       # 4 t-chunks (and attention q-chunks)
VW = 2 * HD          # V1 per-head stride (64 data + 64 ones columns; the
                     # ones columns replicate the softmax denominator into
                     # PSUM partitions 64..127 of the AV output, so the
                     # normalize needs no partition_broadcast)

_NC_CACHE = {}


def _build():
    import concourse.bacc as bacc
    import concourse.mybir as mybir
    import concourse.tile as tile

    f32 = mybir.dt.float32
    f32r = mybir.dt.float32r
    bf16 = mybir.dt.bfloat16
    Exp = mybir.ActivationFunctionType.Exp
    Ident = mybir.ActivationFunctionType.Identity
    Copy = mybir.ActivationFunctionType.Copy

    nc = bacc.Bacc("TRN2", num_devices=N_CORES)

    xT_d = nc.dram_tensor("xT", [C, T], bf16, kind="ExternalInput")
    # merged [wq | wk] columns, pre-transposed: [C, 2*DG]; wv separate
    wqk_d = nc.dram_tensor("wqk", [C, 2 * DG], bf16, kind="ExternalInput")
    wv_d = nc.dram_tensor("wv", [C, DG], bf16, kind="ExternalInput")
    # packed biases: cols = [bq jh0, bq jh1, bk jh0, bk jh1]
    bqk_d = nc.dram_tensor("bqk", [128, 4], f32, kind="ExternalInput")
    bv_d = nc.dram_tensor("bv", [1, DG], f32, kind="ExternalInput")
    # w_proj.T rows [lo:lo+DG] -> partial projection stationary [DG, C]
    wp_d = nc.dram_tensor("wpT", [DG, C], bf16, kind="ExternalInput")
    mask_d = nc.dram_tensor("mask", [128, 128], bf16, kind="ExternalInput")
    oP_d = nc.dram_tensor("oP", [C, T], bf16, kind="ExternalOutput")

    with tile.TileContext(nc) as tc:
        with (
            tc.tile_pool(name="persist", bufs=1) as persist,
            tc.tile_pool(name="xp", bufs=1) as xp,
            tc.tile_pool(name="wp_s", bufs=1) as wp_s,
            tc.tile_pool(name="psp", bufs=1, space="PSUM") as psp,
            tc.tile_pool(name="ppool", bufs=1) as ppool,
            tc.tile_pool(name="npool", bufs=1) as npool,
            tc.tile_pool(name="ynp", bufs=1) as ynp,
            tc.tile_pool(name="otp", bufs=1) as otp,
        ):
            # ---- persistent SBUF ----
            QT = [[persist.tile([128, 512], f32r, name=f"qt{t}_{j}")
                   for j in range(2)] for t in range(NTC)]
            KT = [[persist.tile([128, 512], f32r, name=f"kt{t}_{j}")
                   for j in range(2)] for t in range(NTC)]
            V1 = [persist.tile([128, HPG * VW], bf16, name=f"v{m}")
                  for m in range(4 * NTC)]
            # t=0,1: one tile per (t, k); t=2,3 merged: one [128,1024] per k
            xT01 = [[xp.tile([128, 512], bf16, name=f"x{t}_{k}")
                     for k in range(KC)] for t in range(2)]
            xT23 = [xp.tile([128, 1024], bf16, name=f"x23_{k}")
                    for k in range(KC)]

            def x_sl(t, k, lo=0, hi=512):
                if t < 2:
                    return xT01[t][k][:, lo:hi]
                return xT23[k][:, 512 * (t - 2) + lo:512 * (t - 2) + hi]
            wqk_sb = [wp_s.tile([128, 2 * DG], bf16, name=f"w{k}")
                      for k in range(KC)]
            wv_sb = [wp_s.tile([128, DG], bf16, name=f"wv{k}")
                     for k in range(KC)]
            wpT_sb = [persist.tile([128, C], bf16, name=f"wp_{k}")
                      for k in range(2)]
            mask_sb = persist.tile([128, 128], bf16, name="mask_sb")
            bqk_sb = persist.tile([128, 4], f32, name="bqk_sb")
            bv_row = persist.tile([1, DG], f32, name="bv_row")
            bv_bc = persist.tile([128, DG], f32, name="bv_bc")

            # ---- prologue: warmup + DMA streaming ----
            wu_a = wp_s.tile([128, 128], bf16, name="wu_a")
            wu_b = wp_s.tile([128, 512], bf16, name="wu_b")
            nc.vector.memset(wu_a[:], 0.5)
            nc.vector.memset(wu_b[:], 0.5)
            for i in range(40):
                wu_ps = psp.tile([128, 512], f32, tag="aux", bufs=2,
                                 name=f"wu{i}")
                nc.tensor.matmul(wu_ps[:], wu_a[:], wu_b[:],
                                 start=True, stop=True)

            # Input DMAs on the two hardware DGE queues (sync + scalar),
            # alternated per chunk, in consumption order.  Each DMA costs
            # ~0.7us of issue time on its engine (descriptor gen per row),
            # so transfers are as wide as possible.
            # DMA-issue instructions BLOCK the issuing engine once the HW
            # queue fills, so the scalar (ACT) queue gets only what it can
            # finish before qkv0's drains need the engine (~19us); the long
            # tail (x1-half, wpT, x2/x3) goes on sync, whose engine has no
            # latency-critical role until the first out-DMAs (~45us).
            nc.scalar.dma_start(bqk_sb[:], bqk_d[:])
            nc.sync.dma_start(mask_sb[:], mask_d[:])
            nc.scalar.dma_start(bv_row[:], bv_d[:])
            nc.gpsimd.partition_broadcast(bv_bc[:], bv_row[:])
            # The scalar (ACT) queue only gets DMAs it can finish issuing
            # before qkv0's drains need the engine; everything later-used
            # goes on sync so att-phase exps are never stuck behind issues.
            for k in range(KC):
                eng = nc.sync if k % 2 == 0 else nc.scalar
                eng2 = nc.scalar if k % 2 == 0 else nc.sync
                eng.dma_start(wqk_sb[k][:], wqk_d[128 * k:128 * (k + 1), :])
                eng2.dma_start(xT01[0][k][:],
                               xT_d[128 * k:128 * (k + 1), 0:512])
            for k in range(KC):
                eng = nc.scalar if k % 2 == 0 else nc.sync
                eng.dma_start(wv_sb[k][:], wv_d[128 * k:128 * (k + 1), :])
            for k in range(KC):
                nc.sync.dma_start(xT01[1][k][:],
                                  xT_d[128 * k:128 * (k + 1), 512:1024])
            for k in range(2):
                nc.sync.dma_start(wpT_sb[k][:], wp_d[128 * k:128 * (k + 1), :])
            for k in range(KC):
                nc.sync.dma_start(xT23[k][:],
                                  xT_d[128 * k:128 * (k + 1), 1024:2048])
            # ones columns of V1 (written once; disjoint from the data cols)
            for m in range(4 * NTC):
                vv = V1[m].rearrange("p (h x) -> p h x", h=HPG)
                nc.vector.memset(vv[:, :, HD:2 * HD], 1.0)

            # ---- qkv building blocks ----
            def qkv_q_group(t, jh, sel, drain_on_act=False):
                """One Q-or-K psum group: 8 MMs + bias drain.
                sel 0 -> Q (w cols [0:DG]), sel 1 -> K (w cols [DG:2DG])."""
                dst = QT if sel == 0 else KT

                def thunk():
                    ps = psp.tile([128, 512], f32, tag="aux", bufs=2,
                                  name=f"qk{t}_{sel}_{jh}")
                    for kc in range(KC):
                        nc.tensor.matmul(
                            ps[:],
                            wqk_sb[kc][:, DG * sel + 128 * jh:
                                       DG * sel + 128 * (jh + 1)],
                            x_sl(t, kc),
                            start=(kc == 0), stop=(kc == KC - 1))
                    bcol = bqk_sb[:, 2 * sel + jh:2 * sel + jh + 1]
                    if drain_on_act:
                        nc.scalar.activation(out=dst[t][jh][:], in_=ps[:],
                                             func=Ident, bias=bcol)
                    else:
                        nc.vector.tensor_scalar_add(dst[t][jh][:], ps[:],
                                                    bcol)
                return thunk

            def qkv_v_group(t, mt):
                def thunk():
                    psv = psp.tile([128, 512], f32, tag="aux", bufs=2,
                                   name=f"vps{t}_{mt}")
                    for kc in range(KC):
                        nc.tensor.matmul(
                            psv[:, 0:DG],
                            x_sl(t, kc, 128 * mt, 128 * (mt + 1)),
                            wv_sb[kc][:],
                            start=(kc == 0), stop=(kc == KC - 1))
                    vv = V1[4 * t + mt].rearrange("p (h x) -> p h x", h=HPG)
                    nc.vector.tensor_add(
                        vv[:, :, 0:HD],
                        psv[:, 0:DG].rearrange("p (h x) -> p h x", h=HPG),
                        bv_bc.rearrange("p (h x) -> p h x", h=HPG))
                return thunk

            def qkv_groups(t, drain_on_act=False):
                gs = [qkv_q_group(t, jh, sel, drain_on_act)
                      for sel in range(2) for jh in range(2)]
                gs += [qkv_v_group(t, mt) for mt in range(4)]
                return gs

            # ---- output projection (one half = 4 eh columns) ----
            def proj_half(cq, yns, ehs, tail=False):
                def thunk():
                    for eh in ehs:
                        po = psp.tile([128, 512], f32, tag="aux", bufs=2,
                                      name=f"po_{cq}_{eh}")
                        nc.tensor.matmul(po[:],
                                         wpT_sb[0][:, 128 * eh:128 * (eh + 1)],
                                         yns[0][:], start=True, stop=False)
                        nc.tensor.matmul(po[:],
                                         wpT_sb[1][:, 128 * eh:128 * (eh + 1)],
                                         yns[1][:], start=False, stop=True)
                        ot = otp.tile([128, 512], bf16, tag="ot", bufs=4,
                                      name=f"ot_{cq}_{eh}")
                        if tail and eh % 2 == 1:
                            nc.scalar.activation(out=ot[:], in_=po[:],
                                                 func=Copy)
                        else:
                            nc.vector.tensor_copy(ot[:], po[:])
                        eng = nc.scalar if (tail and eh % 2 == 0) else nc.sync
                        eng.dma_start(
                            oP_d[128 * eh:128 * (eh + 1),
                                 512 * cq:512 * (cq + 1)], ot[:])
                return thunk

            # ---- attention phase with interleaved fillers ----
            def att(cq, fillers_h, end_frac=0.72):
                """Flash attention for q-chunk cq.  fillers_h = (h0, h1):
                per-p-half filler lists, each ceil-paced over the first
                end_frac of that half's kt slots (so the Vector FIFO drains
                before the normalize at the end of each half)."""
                nkt = 4 * (cq + 1)
                win = max(1, int(round(end_frac * nkt)))
                yns = []
                for p in range(2):
                    fl = fillers_h[p]
                    nfl = len(fl)
                    emitted = 0
                    yps = [psp.tile([128, 512], f32, tag=f"y{X}", bufs=1,
                                    name=f"y_{cq}_{p}_{X}") for X in range(2)]

                    def emit_av(kt, Pt, qs):
                        for X in range(2):
                            h = 2 * p + X
                            nc.tensor.matmul(
                                yps[X][:, qs:512],
                                V1[kt][:, VW * h:VW * (h + 1)],
                                Pt[:, 512 * X + qs:512 * (X + 1)],
                                start=(kt == 0), stop=(kt == nkt - 1))

                    pend = None   # AV runs one k-tile behind S/exp
                    for kt in range(nkt):
                        qs = max(0, 128 * kt - 512 * cq)
                        qs2 = min(qs, 256)   # keep f32r free dim >= 256
                        S = psp.tile([128, 1024], f32, tag="s", bufs=2,
                                     name=f"s_{cq}_{p}_{kt}")
                        for X in range(2):
                            nc.tensor.matmul(
                                S[:, 512 * X + qs2:512 * (X + 1)],
                                KT[kt // 4][p][64 * X:64 * (X + 1),
                                               128 * (kt % 4):128 * (kt % 4 + 1)],
                                QT[cq][p][64 * X:64 * (X + 1), qs2:512],
                                start=True, stop=True)
                        if pend is not None:
                            emit_av(*pend)
                        Pt = ppool.tile([128, 1024], bf16, tag="p", bufs=6,
                                        name=f"p_{cq}_{p}_{kt}")
                        nc.scalar.activation(
                            out=Pt.rearrange("pp (x q) -> pp x q",
                                             x=2)[:, :, qs:512],
                            in_=S.rearrange("pp (x q) -> pp x q",
                                            x=2)[:, :, qs:512],
                            func=Exp, scale=1.0 / math.sqrt(HD))
                        if kt >= 4 * cq:  # diagonal block: causal mask
                            for X in range(2):
                                nc.gpsimd.tensor_mul(
                                    Pt[:, 512 * X + qs:512 * X + qs + 128],
                                    Pt[:, 512 * X + qs:512 * X + qs + 128],
                                    mask_sb[:])
                        pend = (kt, Pt, qs)
                        if nfl:
                            tgt = min(nfl,
                                      int(math.ceil(nfl * (kt + 1) / win)))
                            while emitted < tgt:
                                fl[emitted]()
                                emitted += 1
                    emit_av(*pend)
                    # normalize: drain psum fast, then recip/broadcast/mul.
                    # High priority so the scheduler doesn't bury these
                    # latency-critical ops behind filler drains in the
                    # engine FIFOs (the y-bank handoff to the next p half
                    # gates the whole attention pipeline).
                    yn = ynp.tile([128, 512], bf16, tag="yn", bufs=4,
                                  name=f"yn_{cq}_{p}")
                    with tc.high_priority(offset=400):
                        for X in range(2):
                            # denominator is replicated in yps rows 64..127;
                            # cross-partition DVE copy down to 0..63, recip
                            # in place (same partitions), multiply aligned.
                            dn = npool.tile([HD, 512], f32, tag="dn", bufs=4,
                                            name=f"dn_{cq}_{p}_{X}")
                            nc.vector.tensor_copy(dn[:], yps[X][HD:2 * HD, :])
                            rc = npool.tile([HD, 512], f32, tag="rc", bufs=4,
                                            name=f"rc_{cq}_{p}_{X}")
                            nc.vector.reciprocal_approx_fast(out=rc[:],
                                                             in_=dn[:])
                            nc.vector.tensor_mul(
                                yn[64 * X:64 * (X + 1), :],
                                yps[X][0:HD, :], rc[:])
                    yns.append(yn)
                    while emitted < nfl:
                        fl[emitted]()
                        emitted += 1
                return yns

            # ---- the pipeline ----
            # qkv(0) before att(0); its Q/K drains go on ScalarE (idle then)
            for g in qkv_groups(0, drain_on_act=True):
                g()

            # PHASE 0: att(0) + qkv(1), delayed into the p=1 half (x1 DMA)
            yns0 = att(0, ([], qkv_groups(1)), end_frac=0.8)
            # PHASE 1: att(1) + proj(0) + Q(2)/K(2)  (x2 lands ~mid-phase)
            f1 = ([proj_half(0, yns0, [0, 1, 2, 3]),
                   proj_half(0, yns0, [4, 5, 6, 7])],
                  [qkv_q_group(2, 0, 0), qkv_q_group(2, 1, 0),
                   qkv_q_group(2, 0, 1), qkv_q_group(2, 1, 1)])
            yns1 = att(1, f1)
            # PHASE 2: att(2) + V(2) + Q(3) + proj(1)
            f2 = ([qkv_v_group(2, mt) for mt in range(4)],
                  [qkv_q_group(3, 0, 0), qkv_q_group(3, 1, 0),
                   proj_half(1, yns1, [0, 1, 2, 3]),
                   proj_half(1, yns1, [4, 5, 6, 7])])
            yns2 = att(2, f2, end_frac=0.67)
            # PHASE 3: att(3) + K(3) + V(3) + proj(2)
            f3 = ([qkv_q_group(3, 0, 1), qkv_q_group(3, 1, 1)]
                  + [qkv_v_group(3, mt) for mt in range(4)],
                  [proj_half(2, yns2, [0, 1, 2, 3]),
                   proj_half(2, yns2, [4, 5, 6, 7])])
            yns3 = att(3, f3, end_frac=0.67)
            # tail: proj(3) with drains/DMA split across engines
            proj_half(3, yns3, [0, 1, 2, 3], tail=True)()
            proj_half(3, yns3, [4, 5, 6, 7], tail=True)()

    nc.finalize()
    return nc


def _get_nc():
    if "nc" not in _NC_CACHE:
        _NC_CACHE["nc"] = _build()
    return _NC_CACHE["nc"]


def kernel(x, w_attn, b_attn, w_proj, b_proj):
    from concourse.bass_utils import run_bass_kernel_spmd

    x = np.asarray(x, dtype=np.float32)
    w_attn = np.asarray(w_attn, dtype=np.float32)
    b_attn = np.asarray(b_attn, dtype=np.float32)
    w_proj = np.asarray(w_proj, dtype=np.float32)
    b_proj = np.asarray(b_proj, dtype=np.float32)

    mask = np.triu(np.ones((128, 128), dtype=np.float32)).copy()
    wpT_full = np.ascontiguousarray(w_proj.T)  # [C_in, C_out]

    in_maps = []
    for c in range(N_CORES):
        b, g = divmod(c, G)
        lo = DG * g
        wq = w_attn[lo:lo + DG, :].T                    # [C, DG]
        wk = w_attn[C + lo:C + lo + DG, :].T
        wv = w_attn[2 * C + lo:2 * C + lo + DG, :].T
        wqk = np.concatenate([wq, wk], axis=1)          # [C, 2*DG]
        bqk = np.stack([b_attn[lo:lo + 128],
                        b_attn[lo + 128:lo + 256],
                        b_attn[C + lo:C + lo + 128],
                        b_attn[C + lo + 128:C + lo + 256]], axis=1)  # [128,4]
        in_maps.append({
            "xT": np.ascontiguousarray(x[b].T).astype(ml_dtypes.bfloat16),
            "wqk": np.ascontiguousarray(wqk).astype(ml_dtypes.bfloat16),
            "wv": np.ascontiguousarray(wv).astype(ml_dtypes.bfloat16),
            "bqk": np.ascontiguousarray(bqk.astype(np.float32)),
            "bv": np.ascontiguousarray(
                b_attn[2 * C + lo:2 * C + lo + DG].reshape(1, DG)
                .astype(np.float32)),
            "wpT": np.ascontiguousarray(wpT_full[lo:lo + DG, :]).astype(ml_dtypes.bfloat16),
            "mask": mask.astype(ml_dtypes.bfloat16),
        })

    global _last_in_maps
    _last_in_maps = in_maps

    nc = _get_nc()
    res = run_bass_kernel_spmd(nc, in_maps, list(range(N_CORES)))

    out = np.empty((B, T, C), dtype=np.float32)
    for b in range(B):
        acc = np.zeros((C, T), dtype=np.float32)
        for g in range(G):
            acc += res.results[4 * b + g]["oP"].astype(np.float32)
        out[b] = acc.T + b_proj
    return out


# revision 34
# speedup vs baseline: 1.7601x; 1.0377x over previous
"""Causal self-attention (B=2, T=2048, C=1024, H=16) on 8 trn2 NeuronCores.

Sharding: core c = (b, g) with b = c // 4 (batch), g = c % 4 (head-group of 4
heads = 256 dims).  No collectives: each core computes a PARTIAL output
projection over its own 256 head-dims (o_part^T = W_g^T y_g^T, bf16) and the
host sums the 4 partials per batch during unsharding.

v3 schedule: the attention phases are ScalarE(exp)-bound, so QKV / output
projection matmul groups are emitted as "fillers" interleaved INTO the
attention kt-loops -- the in-order PE queue always has independent ready work
while exp lags.  Fillers are paced away from the p-boundaries so the Vector
FIFO drains before each normalize (whose PSUM reads gate the y-bank handoff
to the next p half).  Input DMAs are merged (w_qkv as one [C,768] tensor,
biases packed) to cut the ~0.7us/instruction descriptor-generation cost, and
alternate between the two hardware DGE queues in consumption order.
"""
import math

import numpy as np
import ml_dtypes

B, T, C, H = 2, 2048, 1024, 16
HD = C // H          # 64 head dim
G = 4                # head-groups (cores per batch)
HPG = H // G         # 4 heads per group
DG = HPG * HD        # 256 dims per group
N_CORES = 8
KC = C // 128        # 8 contraction chunks
NTC = T // 512       # 4 t-chunks (and attention q-chunks)
VW = 2 * HD          # V1 per-head stride (64 data + 64 ones columns; the
                     # ones columns replicate the softmax denominator into
                     # PSUM partitions 64..127 of the AV output, so the
                     # normalize needs no partition_broadcast)

_NC_CACHE = {}


def _build():
    import concourse.bacc as bacc
    import concourse.mybir as mybir
    import concourse.tile as tile

    f32 = mybir.dt.float32
    f32r = mybir.dt.float32r
    bf16 = mybir.dt.bfloat16
    Exp = mybir.ActivationFunctionType.Exp
    Ident = mybir.ActivationFunctionType.Identity
    Copy = mybir.ActivationFunctionType.Copy

    nc = bacc.Bacc("TRN2", num_devices=N_CORES)

    xT_d = nc.dram_tensor("xT", [C, T], bf16, kind="ExternalInput")
    # merged [wq | wk] columns, pre-transposed: [C, 2*DG]; wv separate
    wqk_d = nc.dram_tensor("wqk", [C, 2 * DG], bf16, kind="ExternalInput")
    wv_d = nc.dram_tensor("wv", [C, DG], bf16, kind="ExternalInput")
    # packed biases: cols = [bq jh0, bq jh1, bk jh0, bk jh1]
    bqk_d = nc.dram_tensor("bqk", [128, 4], f32, kind="ExternalInput")
    bv_d = nc.dram_tensor("bv", [1, DG], f32, kind="ExternalInput")
    # w_proj.T rows [lo:lo+DG] -> partial projection stationary [DG, C]
    wp_d = nc.dram_tensor("wpT", [DG, C], bf16, kind="ExternalInput")
    mask_d = nc.dram_tensor("mask", [128, 128], bf16, kind="ExternalInput")
    oP_d = nc.dram_tensor("oP", [C, T], bf16, kind="ExternalOutput")

    with tile.TileContext(nc) as tc:
        with (
            tc.tile_pool(name="persist", bufs=1) as persist,
            tc.tile_pool(name="xp", bufs=1) as xp,
            tc.tile_pool(name="wp_s", bufs=1) as wp_s,
            tc.tile_pool(name="psp", bufs=1, space="PSUM") as psp,
            tc.tile_pool(name="ppool", bufs=1) as ppool,
            tc.tile_pool(name="npool", bufs=1) as npool,
            tc.tile_pool(name="ynp", bufs=1) as ynp,
            tc.tile_pool(name="otp", bufs=1) as otp,
        ):
            # ---- persistent SBUF ----
            QT = [[persist.tile([128, 512], bf16, name=f"qt{t}_{j}")
                   for j in range(2)] for t in range(NTC)]
            KT = [[persist.tile([128, 512], bf16, name=f"kt{t}_{j}")
                   for j in range(2)] for t in range(NTC)]
            V1 = [persist.tile([128, HPG * VW], bf16, name=f"v{m}")
                  for m in range(4 * NTC)]
            # t=0,1: one tile per (t, k); t=2,3 merged: one [128,1024] per k
            xT01 = [[xp.tile([128, 512], bf16, name=f"x{t}_{k}")
                     for k in range(KC)] for t in range(2)]
            xT23 = [xp.tile([128, 1024], bf16, name=f"x23_{k}")
                    for k in range(KC)]

            def x_sl(t, k, lo=0, hi=512):
                if t < 2:
                    return xT01[t][k][:, lo:hi]
                return xT23[k][:, 512 * (t - 2) + lo:512 * (t - 2) + hi]
            wqk_sb = [wp_s.tile([128, 2 * DG], bf16, name=f"w{k}")
                      for k in range(KC)]
            wv_sb = [wp_s.tile([128, DG], bf16, name=f"wv{k}")
                     for k in range(KC)]
            wpT_sb = [persist.tile([128, C], bf16, name=f"wp_{k}")
                      for k in range(2)]
            mask_sb = persist.tile([128, 128], bf16, name="mask_sb")
            bqk_sb = persist.tile([128, 4], f32, name="bqk_sb")
            bv_row = persist.tile([1, DG], f32, name="bv_row")
            bv_bc = persist.tile([128, DG], f32, name="bv_bc")

            # ---- prologue: warmup + DMA streaming ----
            wu_a = wp_s.tile([128, 128], bf16, name="wu_a")
            wu_b = wp_s.tile([128, 512], bf16, name="wu_b")
            nc.vector.memset(wu_a[:], 0.5)
            nc.vector.memset(wu_b[:], 0.5)
            for i in range(20):
                wu_ps = psp.tile([128, 512], f32, tag="aux", bufs=2,
                                 name=f"wu{i}")
                nc.tensor.matmul(wu_ps[:], wu_a[:], wu_b[:],
                                 start=True, stop=True)

            # Input DMAs on the two hardware DGE queues (sync + scalar),
            # alternated per chunk, in consumption order.  Each DMA costs
            # ~0.7us of issue time on its engine (descriptor gen per row),
            # so transfers are as wide as possible.
            # DMA-issue instructions BLOCK the issuing engine once the HW
            # queue fills, so the scalar (ACT) queue gets only what it can
            # finish before qkv0's drains need the engine (~19us); the long
            # tail (x1-half, wpT, x2/x3) goes on sync, whose engine has no
            # latency-critical role until the first out-DMAs (~45us).
            nc.scalar.dma_start(bqk_sb[:], bqk_d[:])
            nc.sync.dma_start(mask_sb[:], mask_d[:])
            nc.scalar.dma_start(bv_row[:], bv_d[:])
            nc.gpsimd.partition_broadcast(bv_bc[:], bv_row[:])
            # The scalar (ACT) queue only gets DMAs it can finish issuing
            # before qkv0's drains need the engine; everything later-used
            # goes on sync so att-phase exps are never stuck behind issues.
            for k in range(KC):
                eng = nc.sync if k % 2 == 0 else nc.scalar
                eng2 = nc.scalar if k % 2 == 0 else nc.sync
                eng.dma_start(wqk_sb[k][:], wqk_d[128 * k:128 * (k + 1), :])
                eng2.dma_start(xT01[0][k][:],
                               xT_d[128 * k:128 * (k + 1), 0:512])
            for k in range(KC):
                eng = nc.scalar if k % 2 == 0 else nc.sync
                eng.dma_start(wv_sb[k][:], wv_d[128 * k:128 * (k + 1), :])
            for k in range(KC):
                nc.sync.dma_start(xT01[1][k][:],
                                  xT_d[128 * k:128 * (k + 1), 512:1024])
            for k in range(2):
                nc.sync.dma_start(wpT_sb[k][:], wp_d[128 * k:128 * (k + 1), :])
            for k in range(KC):
                nc.sync.dma_start(xT23[k][:],
                                  xT_d[128 * k:128 * (k + 1), 1024:2048])
            # ones columns of V1 (written once; disjoint from the data cols)
            for m in range(4 * NTC):
                vv = V1[m].rearrange("p (h x) -> p h x", h=HPG)
                nc.vector.memset(vv[:, :, HD:2 * HD], 1.0)

            # ---- qkv building blocks ----
            def qkv_q_group(t, jh, sel, drain_on_act=False):
                """One Q-or-K psum group: 8 MMs + bias drain.
                sel 0 -> Q (w cols [0:DG]), sel 1 -> K (w cols [DG:2DG])."""
                dst = QT if sel == 0 else KT

                def thunk():
                    ps = psp.tile([128, 512], f32, tag="aux", bufs=2,
                                  name=f"qk{t}_{sel}_{jh}")
                    for kc in range(KC):
                        nc.tensor.matmul(
                            ps[:],
                            wqk_sb[kc][:, DG * sel + 128 * jh:
                                       DG * sel + 128 * (jh + 1)],
                            x_sl(t, kc),
                            start=(kc == 0), stop=(kc == KC - 1))
                    bcol = bqk_sb[:, 2 * sel + jh:2 * sel + jh + 1]
                    if drain_on_act:
                        nc.scalar.activation(out=dst[t][jh][:], in_=ps[:],
                                             func=Ident, bias=bcol)
                    else:
                        nc.vector.tensor_scalar_add(dst[t][jh][:], ps[:],
                                                    bcol)
                return thunk

            def qkv_v_group(t, mt):
                def thunk():
                    psv = psp.tile([128, 512], f32, tag="aux", bufs=2,
                                   name=f"vps{t}_{mt}")
                    for kc in range(KC):
                        nc.tensor.matmul(
                            psv[:, 0:DG],
                            x_sl(t, kc, 128 * mt, 128 * (mt + 1)),
                            wv_sb[kc][:],
                            start=(kc == 0), stop=(kc == KC - 1))
                    vv = V1[4 * t + mt].rearrange("p (h x) -> p h x", h=HPG)
                    nc.vector.tensor_add(
                        vv[:, :, 0:HD],
                        psv[:, 0:DG].rearrange("p (h x) -> p h x", h=HPG),
                        bv_bc.rearrange("p (h x) -> p h x", h=HPG))
                return thunk

            def qkv_groups(t, drain_on_act=False):
                gs = [qkv_q_group(t, jh, sel, drain_on_act)
                      for sel in range(2) for jh in range(2)]
                gs += [qkv_v_group(t, mt) for mt in range(4)]
                return gs

            # ---- output projection (one half = 4 eh columns) ----
            def proj_half(cq, yns, ehs, tail=False):
                def thunk():
                    for eh in ehs:
                        po = psp.tile([128, 512], f32, tag="aux", bufs=2,
                                      name=f"po_{cq}_{eh}")
                        nc.tensor.matmul(po[:],
                                         wpT_sb[0][:, 128 * eh:128 * (eh + 1)],
                                         yns[0][:], start=True, stop=False)
                        nc.tensor.matmul(po[:],
                                         wpT_sb[1][:, 128 * eh:128 * (eh + 1)],
                                         yns[1][:], start=False, stop=True)
                        ot = otp.tile([128, 512], bf16, tag="ot", bufs=4,
                                      name=f"ot_{cq}_{eh}")
                        if tail and eh % 2 == 1:
                            nc.scalar.activation(out=ot[:], in_=po[:],
                                                 func=Copy)
                        else:
                            nc.vector.tensor_copy(ot[:], po[:])
                        eng = nc.scalar if (tail and eh % 2 == 0) else nc.sync
                        eng.dma_start(
                            oP_d[128 * eh:128 * (eh + 1),
                                 512 * cq:512 * (cq + 1)], ot[:])
                return thunk

            # ---- attention phase with interleaved fillers ----
            def att(cq, fillers_h, end_frac=0.72):
                """Flash attention for q-chunk cq.  fillers_h = (h0, h1):
                per-p-half filler lists, each ceil-paced over the first
                end_frac of that half's kt slots (so the Vector FIFO drains
                before the normalize at the end of each half)."""
                nkt = 4 * (cq + 1)
                win = max(1, int(round(end_frac * nkt)))
                yns = []
                for p in range(2):
                    fl = fillers_h[p]
                    nfl = len(fl)
                    emitted = 0
                    yps = [psp.tile([128, 512], f32, tag=f"y{X}", bufs=1,
                                    name=f"y_{cq}_{p}_{X}") for X in range(2)]

                    def emit_av(kt, Pt, qs):
                        for X in range(2):
                            h = 2 * p + X
                            nc.tensor.matmul(
                                yps[X][:, qs:512],
                                V1[kt][:, VW * h:VW * (h + 1)],
                                Pt[:, 512 * X + qs:512 * (X + 1)],
                                start=(kt == 0), stop=(kt == nkt - 1))

                    pend = None   # AV runs one k-tile behind S/exp
                    for kt in range(nkt):
                        qs = max(0, 128 * kt - 512 * cq)
                        qs2 = qs
                        S = psp.tile([128, 1024], f32, tag="s", bufs=2,
                                     name=f"s_{cq}_{p}_{kt}")
                        for X in range(2):
                            nc.tensor.matmul(
                                S[:, 512 * X + qs2:512 * (X + 1)],
                                KT[kt // 4][p][64 * X:64 * (X + 1),
                                               128 * (kt % 4):128 * (kt % 4 + 1)],
                                QT[cq][p][64 * X:64 * (X + 1), qs2:512],
                                start=True, stop=True)
                        if pend is not None:
                            emit_av(*pend)
                        Pt = ppool.tile([128, 1024], bf16, tag="p", bufs=6,
                                        name=f"p_{cq}_{p}_{kt}")
                        nc.scalar.activation(
                            out=Pt.rearrange("pp (x q) -> pp x q",
                                             x=2)[:, :, qs:512],
                            in_=S.rearrange("pp (x q) -> pp x q",
                                            x=2)[:, :, qs:512],
                            func=Exp, scale=1.0 / math.sqrt(HD))
                        if kt >= 4 * cq:  # diagonal block: causal mask
                            for X in range(2):
                                nc.gpsimd.tensor_mul(
                                    Pt[:, 512 * X + qs:512 * X + qs + 128],
                                    Pt[:, 512 * X + qs:512 * X + qs + 128],
                                    mask_sb[:])
                        pend = (kt, Pt, qs)
                        if nfl:
                            tgt = min(nfl,
                                      int(math.ceil(nfl * (kt + 1) / win)))
                            while emitted < tgt:
                                fl[emitted]()
                                emitted += 1
                    emit_av(*pend)
                    # normalize: drain psum fast, then recip/broadcast/mul.
                    # High priority so the scheduler doesn't bury these
                    # latency-critical ops behind filler drains in the
                    # engine FIFOs (the y-bank handoff to the next p half
                    # gates the whole attention pipeline).
                    yn = ynp.tile([128, 512], bf16, tag="yn", bufs=4,
                                  name=f"yn_{cq}_{p}")
                    with tc.high_priority(offset=400):
                        for X in range(2):
                            # denominator is replicated in yps rows 64..127;
                            # cross-partition DVE copy down to 0..63, recip
                            # in place (same partitions), multiply aligned.
                            dn = npool.tile([HD, 512], f32, tag="dn", bufs=4,
                                            name=f"dn_{cq}_{p}_{X}")
                            nc.vector.tensor_copy(dn[:], yps[X][HD:2 * HD, :])
                            rc = npool.tile([HD, 512], f32, tag="rc", bufs=4,
                                            name=f"rc_{cq}_{p}_{X}")
                            nc.vector.reciprocal_approx_fast(out=rc[:],
                                                             in_=dn[:])
                            nc.vector.tensor_mul(
                                yn[64 * X:64 * (X + 1), :],
                                yps[X][0:HD, :], rc[:])
                    yns.append(yn)
                    while emitted < nfl:
                        fl[emitted]()
                        emitted += 1
                return yns

            # ---- the pipeline ----
            # qkv(0) before att(0); its Q/K drains go on ScalarE (idle then)
            for g in qkv_groups(0, drain_on_act=True):
                g()

            # PHASE 0: att(0) + qkv(1), delayed into the p=1 half (x1 DMA)
            yns0 = att(0, ([], qkv_groups(1)), end_frac=0.8)
            # PHASE 1: att(1) + proj(0) + Q(2)/K(2)  (x2 lands ~mid-phase)
            f1 = ([proj_half(0, yns0, [0, 1, 2, 3]),
                   proj_half(0, yns0, [4, 5, 6, 7])],
                  [qkv_q_group(2, 0, 0), qkv_q_group(2, 1, 0),
                   qkv_q_group(2, 0, 1), qkv_q_group(2, 1, 1)])
            yns1 = att(1, f1)
            # PHASE 2: att(2) + V(2) + Q(3) + proj(1)
            f2 = ([qkv_v_group(2, mt) for mt in range(4)],
                  [qkv_q_group(3, 0, 0), qkv_q_group(3, 1, 0),
                   proj_half(1, yns1, [0, 1, 2, 3]),
                   proj_half(1, yns1, [4, 5, 6, 7])])
            yns2 = att(2, f2, end_frac=0.67)
            # PHASE 3: att(3) + K(3) + V(3) + proj(2)
            f3 = ([qkv_q_group(3, 0, 1), qkv_q_group(3, 1, 1)]
                  + [qkv_v_group(3, mt) for mt in range(4)],
                  [proj_half(2, yns2, [0, 1, 2, 3]),
                   proj_half(2, yns2, [4, 5, 6, 7])])
            yns3 = att(3, f3, end_frac=0.67)
            # tail: proj(3) with drains/DMA split across engines
            proj_half(3, yns3, [0, 1, 2, 3], tail=True)()
            proj_half(3, yns3, [4, 5, 6, 7], tail=True)()

    nc.finalize()
    return nc


def _get_nc():
    if "nc" not in _NC_CACHE:
        _NC_CACHE["nc"] = _build()
    return _NC_CACHE["nc"]


def kernel(x, w_attn, b_attn, w_proj, b_proj):
    from concourse.bass_utils import run_bass_kernel_spmd

    x = np.asarray(x, dtype=np.float32)
    w_attn = np.asarray(w_attn, dtype=np.float32)
    b_attn = np.asarray(b_attn, dtype=np.float32)
    w_proj = np.asarray(w_proj, dtype=np.float32)
    b_proj = np.asarray(b_proj, dtype=np.float32)

    mask = np.triu(np.ones((128, 128), dtype=np.float32)).copy()
    wpT_full = np.ascontiguousarray(w_proj.T)  # [C_in, C_out]

    in_maps = []
    for c in range(N_CORES):
        b, g = divmod(c, G)
        lo = DG * g
        wq = w_attn[lo:lo + DG, :].T                    # [C, DG]
        wk = w_attn[C + lo:C + lo + DG, :].T
        wv = w_attn[2 * C + lo:2 * C + lo + DG, :].T
        wqk = np.concatenate([wq, wk], axis=1)          # [C, 2*DG]
        bqk = np.stack([b_attn[lo:lo + 128],
                        b_attn[lo + 128:lo + 256],
                        b_attn[C + lo:C + lo + 128],
                        b_attn[C + lo + 128:C + lo + 256]], axis=1)  # [128,4]
        in_maps.append({
            "xT": np.ascontiguousarray(x[b].T).astype(ml_dtypes.bfloat16),
            "wqk": np.ascontiguousarray(wqk).astype(ml_dtypes.bfloat16),
            "wv": np.ascontiguousarray(wv).astype(ml_dtypes.bfloat16),
            "bqk": np.ascontiguousarray(bqk.astype(np.float32)),
            "bv": np.ascontiguousarray(
                b_attn[2 * C + lo:2 * C + lo + DG].reshape(1, DG)
                .astype(np.float32)),
            "wpT": np.ascontiguousarray(wpT_full[lo:lo + DG, :]).astype(ml_dtypes.bfloat16),
            "mask": mask.astype(ml_dtypes.bfloat16),
        })

    global _last_in_maps
    _last_in_maps = in_maps

    nc = _get_nc()
    res = run_bass_kernel_spmd(nc, in_maps, list(range(N_CORES)))

    out = np.empty((B, T, C), dtype=np.float32)
    for b in range(B):
        acc = np.zeros((C, T), dtype=np.float32)
        for g in range(G):
            acc += res.results[4 * b + g]["oP"].astype(np.float32)
        out[b] = acc.T + b_proj
    return out
